# revision 1
# baseline (speedup 1.0000x reference)
"""Trainium2 Bass kernel for nn_DSSMReverse (DSSM embed/conv/VQ/Gram model).

Strategy: data-parallel over batch across 8 NeuronCores (128 images each).
 - Embedding+conv0 are fully composed on host: 9-tap-shifted embedding
   planes (72 = 8ch x 9 taps) make conv0 a single K=72 matmul per image.
 - Remaining 3x3 convs run as shifted-window matmuls with row-group
   packing (K=96/128+64); relu/shift copies are spread across the
   ACT/DVE/Pool engines; emission is stage-major with phi staggering so
   the PE rarely waits on the relu copies.
 - phi conv outputs [128c x 400px] spill to DRAM in pixel-group-major
   layout; the 51200->512 linears stream x tiles [128,128,100] and
   weight tiles [128,10,512] (both contiguous) from HBM. Weights can be
   fp8-e4m3 per phi (scale 128, descaled in the bias epilogue).
 - Phase C (feat-major [4x128, B], fp16): VQ codebook argmax+gather via
   matmuls + PE transposes, fc stacks, L2 normalize via ones-matmul,
   AllGather of sp_out, per-core [128, 1024] block of the Gram matrix.
"""

import os
import numpy as np
import ml_dtypes

BF_NP = np.float16
F8_NP = ml_dtypes.float8_e4m3

import concourse.bacc as bacc
import concourse.bass as bass
import concourse.mybir as mybir
import concourse.tile as tile
from concourse.masks import make_identity

N_CORES = 8
B_FULL = 1024
H = W = 20
PW = 22            # padded plane width
NPIX = 400
NPAD = 484         # 22*22
EPS = 1e-4
F32 = mybir.dt.float32
BF16 = mybir.dt.float16    # 16-bit matmul dtype (fp16: 1 cyc/row)
FP8 = mybir.dt.float8e4
AF = mybir.ActivationFunctionType
ALU = mybir.AluOpType

W8 = (0,)          # phis whose big-linear weights are fp8 (0=p1,1=p2,2=p3)
                   # p1 is safe (2e-3, VQ-diluted); p2 flips VQ indices
                   # (catastrophic); p3 costs ~1.6e-2 (thin margin)
X8 = ()            # conv features stay fp16: fp8-x only pays off on a
                   # DMA-bound phi, and the only fp8-safe phi (p1) is
                   # already PE-bound
F8_SCALE = 128.0
W_PIX = 10         # pixels per streamed weight tile in phase B
G_PIX = 100        # pixels per x tile in phase B


# --------------------------------------------------------------------------
# device program
# --------------------------------------------------------------------------

_STAGE_LOG = []            # (label, first-instruction-name) when tracing


def build_program(BL, phases="ABC", ib=2, sim1=False, trace_stages=False):
    nc = bacc.Bacc("TRN2", target_bir_lowering=False, debug=False,
                   num_devices=1 if sim1 else N_CORES)
    _STAGE_LOG.clear()

    def mark(label):
        if trace_stages:
            _STAGE_LOG.append((label, nc.get_next_instruction_name()))

    def inp(name, shape, dt=F32):
        return nc.dram_tensor(name, shape, dt, kind="ExternalInput").ap()

    embp_s = inp("embp_s", [BL, 72, NPAD], BF16)
    embp_sp = inp("embp_sp", [BL, 72, NPAD], BF16)
    wE0 = inp("wE0", [72, 32], BF16)
    wE1 = inp("wE1", [96, 3, 64], BF16)
    wA0 = inp("wA0", [128, 9, 64], BF16)
    wB0 = inp("wB0", [128, 9, 64], BF16)
    wA1 = inp("wA1", [128, 9, 128], BF16)
    wB1 = inp("wB1", [128, 9, 128], BF16)
    b0 = inp("b0", [32, 1])
    b1 = inp("b1", [64, 1])
    bc0 = inp("bc0", [64, 3])
    bc1 = inp("bc1", [128, 3])
    NWT = NPIX // W_PIX
    wl = [inp(f"wl{p}", [NWT, 128, W_PIX, 512], FP8 if p in W8 else BF16)
          for p in range(3)]
    blpk = inp("blpk", [128, 3, 4])
    fcT = {n: inp(n + "T", [128, 4, 512], BF16)
           for n in ("fc0", "fc1", "p3f0", "p3f1")}
    fcB = {n: inp(n + "b", [128, 4]) for n in ("fc0", "fc1", "p3f0", "p3f1")}
    zT = inp("zT", [128, 4, 64], BF16)
    zrow = inp("zrow", [64, 512], BF16)
    zsq = inp("zsq", [64, 1])
    esc = inp("esc", [1, 1])

    out_d = nc.dram_tensor("out", [BL, N_CORES * BL], F32,
                           kind="ExternalOutput").ap()

    cpc = min(N_CORES, 512 // BL)          # gram col-cores per matmul chunk
    n_chunks = (N_CORES + cpc - 1) // cpc
    NG = NPIX // G_PIX                     # x pixel groups in phase B

    with tile.TileContext(nc) as tc:
        with (
            tc.tile_pool(name="const", bufs=1) as cst,
            tc.tile_pool(name="inter", bufs=1) as inter,
            tc.tile_pool(name="dram", bufs=1, space="DRAM") as dram,
        ):
            # ---- persistent weights -> SBUF
            def load(ap, shape, tag, dt=F32):
                t = cst.tile(shape, dt, tag=tag, name=tag)
                nc.sync.dma_start(out=t[:], in_=ap[:])
                return t

            wE0_s = load(wE0, [72, 32], "wE0", BF16)
            wE1_s = load(wE1, [96, 3, 64], "wE1", BF16)
            wA0_s = load(wA0, [128, 9, 64], "wA0", BF16)
            wB0_s = load(wB0, [128, 9, 64], "wB0", BF16)
            wA1_s = load(wA1, [128, 9, 128], "wA1", BF16)
            wB1_s = load(wB1, [128, 9, 128], "wB1", BF16)
            b0_s = load(b0, [32, 1], "b0")
            b1_s = load(b1, [64, 1], "b1")
            bc0_s = load(bc0, [64, 3], "bc0")
            bc1_s = load(bc1, [128, 3], "bc1")
            bl_s = load(blpk, [128, 3, 4], "blpk")
            # phase-C-only weights: demote so they don't block the first
            # embed-plane DMAs on the sync queue at kernel start
            with tc.high_priority(offset=-100000):
                fcT_s = {n: load(fcT[n], [128, 4, 512], n + "T", BF16)
                         for n in fcT}
                fcB_s = {n: load(fcB[n], [128, 4], n + "b") for n in fcB}
                zT_s = load(zT, [128, 4, 64], "zT", BF16)
                zrow_s = load(zrow, [64, 512], "zrow", BF16)
                zsq_s = load(zsq, [64, 1], "zsq")
                esc_s = load(esc, [1, 1], "esc")

            ident = cst.tile([128, 128], F32, tag="ident")
            make_identity(nc, ident[:])
            ones_col = cst.tile([128, 1], F32, tag="ones_col")
            nc.vector.memset(ones_col[:], 1.0)
            ones_row = cst.tile([1, 128], F32, tag="ones_row")
            nc.vector.memset(ones_row[:], 1.0)

            # DRAM spill for phi conv1 outputs, [c, pxgroup, img, px_in_g]
            xdt = [FP8 if i in X8 else BF16 for i in range(3)]
            xsp = [dram.tile([128, NG, BL, G_PIX], xdt[i], tag=f"xsp{i}",
                             name=f"xsp{i}") for i in range(3)]
            # collective buffers
            spT_d = dram.tile([128, 4, BL], BF16, tag="spT")
            spall = dram.tile([N_CORES, 128, 4, BL], BF16, tag="spall",
                              **({} if sim1 else {"addr_space": "Shared"}))

            # ============================================================
            # phase A: conv stacks, IB images per instruction group (fp16)
            # ============================================================
            IB = ib if BL % ib == 0 else 1
            with (
                tc.tile_pool(name="pa", bufs=2) as pa,
                tc.tile_pool(name="papsh", bufs=2, space="PSUM") as papsh,
                tc.tile_pool(name="papsb", bufs=2, space="PSUM") as papsb,
            ):
                def win(t4, i, y0, y1, kx):
                    """conv window of image i: [P, 20, 20] at row y0..y1-1."""
                    v = t4[:, i, :].rearrange("p (y x) -> p y x", y=PW, x=PW)
                    return v[:, y0:y1, kx:kx + 20]

                # rotating engine choice for relu copies (Pool is ~1.3x
                # slower per element, so it gets a smaller share)
                ENG = (nc.scalar, nc.vector, nc.gpsimd)

                def relu_copy(eng, dst, src, bias):
                    if eng is nc.scalar:
                        eng.activation(dst, src, AF.Relu, bias=bias)
                    else:
                        eng.tensor_scalar(dst, src, bias, 0.0, ALU.add,
                                          ALU.max)

                def dup_copy(dst, src):
                    # Pool cannot read PSUM: shifted duplicates are plain
                    # SBUF->SBUF copies of already-relu'd data
                    nc.gpsimd.tensor_copy(dst, src)

                stash = {}   # group idx -> dict of live tiles

                def dma_head(g):
                    mark(f'dma_head:{g}')
                    st = stash.setdefault(g, {})
                    g0 = g * IB
                    for m, ap in (("s", embp_s), ("sp", embp_sp)):
                        t = pa.tile([72, IB, NPAD], BF16, tag=f"embp_{m}",
                                    bufs=3)
                        nc.sync.dma_start(
                            out=t[:],
                            in_=ap[g0:g0 + IB].rearrange("i p n -> p i n"))
                        st[f"embp_{m}"] = t

                def head_e0(g):
                    # E0 matmuls (K=72, 9 taps host-composed) both tensors
                    # into one psum [64,...] + 3 row-shifted relu copies
                    mark(f'head_e0:{g}')
                    st = stash[g]
                    g0 = g * IB
                    ps = papsh.tile([64, IB, 512], F32, tag="ps")
                    for mi, m in enumerate(("s", "sp")):
                        for i in range(IB):
                            v = st[f"embp_{m}"][:, i, :].rearrange(
                                "p (y x) -> p y x", y=PW, x=PW)
                            nc.tensor.matmul(
                                ps[mi * 32:(mi + 1) * 32, i, 0:NPIX],
                                wE0_s[:], v[:, 1:21, 1:21],
                                start=True, stop=True)
                    for mi, m in enumerate(("s", "sp")):
                        eb = pa.tile([96, IB, NPAD], BF16, tag="e0big",
                                     bufs=3)
                        if g0 < 3 * IB:
                            nc.gpsimd.memset(eb[:], 0.0)
                        ev = eb[:].rearrange("p i (y x) -> p i y x",
                                             y=PW, x=PW)
                        pv = ps[mi * 32:(mi + 1) * 32, :, 0:NPIX].rearrange(
                            "p i (y x) -> p i y x", y=20, x=20)
                        relu_copy(nc.scalar, ev[0:32, :, 1:21, 1:21], pv[:],
                                  b0_s[:])
                        relu_copy(nc.vector, ev[32:64, :, 0:20, 1:21], pv[:],
                                  b0_s[:])
                        dup_copy(ev[64:96, :, 0:19, 1:21],
                                 ev[0:32, :, 2:21, 1:21])
                        st[f"e0big_{m}"] = eb

                def head_e1(g):
                    # E1 matmuls (K=96, 3 kx taps) both tensors into one
                    # psum [128,...] + embbig relus + diff
                    mark(f'head_e1:{g}')
                    st = stash[g]
                    g0 = g * IB
                    ps = papsh.tile([128, IB, 512], F32, tag="ps")
                    for mi, m in enumerate(("s", "sp")):
                        for kx in range(3):
                            for i in range(IB):
                                nc.tensor.matmul(
                                    ps[mi * 64:(mi + 1) * 64, i, 0:NPIX],
                                    wE1_s[:, kx, :],
                                    win(st[f"e0big_{m}"], i, 0, 20, kx),
                                    start=(kx == 0), stop=(kx == 2))
                    for mi, m in enumerate(("s", "sp")):
                        eb = pa.tile([128, IB, NPAD], BF16, tag="emb",
                                     bufs=4)
                        if g0 < 4 * IB:
                            nc.gpsimd.memset(eb[:], 0.0)
                        ebv = eb[:].rearrange("p i (y x) -> p i y x",
                                              y=PW, x=PW)
                        p1v = ps[mi * 64:(mi + 1) * 64, :, 0:NPIX].rearrange(
                            "p i (y x) -> p i y x", y=20, x=20)
                        relu_copy(nc.scalar if mi == 0 else nc.vector,
                                  ebv[0:64, :, 1:21, 1:21], p1v[:], b1_s[:])
                        dup_copy(ebv[64:128, :, 0:20, 1:21],
                                 ebv[0:64, :, 1:21, 1:21])
                        st[f"emb_{m}"] = eb
                    diff = pa.tile([128, IB, NPAD], BF16, tag="diff",
                                   bufs=2)
                    nc.vector.tensor_tensor(diff[:], st["emb_sp"][:],
                                            st["emb_s"][:], ALU.subtract)
                    st["diff"] = diff

                def c0_mm(g, phi):
                    mark(f'c0_mm:{g}:{phi}')
                    st = stash[g]
                    src = {0: st["emb_s"], 1: st["diff"],
                           2: st["emb_sp"]}[phi]
                    ps = papsb.tile([64, IB, 512], F32, tag="ps")
                    for i in range(IB):
                        for kx in range(3):
                            nc.tensor.matmul(
                                ps[:, i, 0:NPIX],
                                wA0_s[:, phi * 3 + kx, :],
                                win(src, i, 0, 20, kx),
                                start=(kx == 0), stop=False)
                        for kx in range(3):
                            nc.tensor.matmul(
                                ps[:, i, 0:NPIX],
                                wB0_s[64:128, phi * 3 + kx, :],
                                win(src, i, 1, 21, kx)[64:128],
                                start=False, stop=(kx == 2))
                    return ps

                def c0_relu(g, phi, ps, k):
                    mark(f'c0_relu:{g}:{phi}')
                    st = stash[g]
                    cb = pa.tile([128, IB, NPAD], BF16, tag="c0big",
                                 bufs=3)
                    if g * IB < 3 * IB:
                        nc.gpsimd.memset(cb[:], 0.0)
                    cv = cb[:].rearrange("p i (y x) -> p i y x", y=PW, x=PW)
                    c0v = ps[:, :, 0:NPIX].rearrange(
                        "p i (y x) -> p i y x", y=20, x=20)
                    relu_copy(ENG[k % 2], cv[0:64, :, 1:21, 1:21], c0v[:],
                              bc0_s[:, phi:phi + 1])
                    if phi == 0:
                        # chain-critical: c1P1 is the next PE burst; drain
                        # PSUM directly on the other vector engine
                        relu_copy(ENG[(k + 1) % 2],
                                  cv[64:128, :, 0:20, 1:21], c0v[:],
                                  bc0_s[:, phi:phi + 1])
                    else:
                        dup_copy(cv[64:128, :, 0:20, 1:21],
                                 cv[0:64, :, 1:21, 1:21])
                    st[f"c0big{phi}"] = cb

                def c1_mm(g, phi):
                    mark(f'c1_mm:{g}:{phi}')
                    st = stash[g]
                    src = st[f"c0big{phi}"]
                    ps = papsb.tile([128, IB, 512], F32, tag="ps")
                    for i in range(IB):
                        for kx in range(3):
                            nc.tensor.matmul(
                                ps[:, i, 0:NPIX],
                                wA1_s[:, phi * 3 + kx, :],
                                win(src, i, 0, 20, kx),
                                start=(kx == 0), stop=False)
                        for kx in range(3):
                            nc.tensor.matmul(
                                ps[:, i, 0:NPIX],
                                wB1_s[64:128, phi * 3 + kx, :],
                                win(src, i, 1, 21, kx)[64:128],
                                start=False, stop=(kx == 2))
                    st[f"psC1_{phi}"] = ps

                def c1_out(g, phi, k):
                    mark(f'c1_out:{g}:{phi}')
                    st = stash[g]
                    g0 = g * IB
                    c1sb = pa.tile([128, NG, IB, G_PIX], xdt[phi],
                                   tag=f"c1sb{xdt[phi]}", bufs=3)
                    # spill copies are off the critical path: demote them
                    # so chain-critical relu copies win engine races
                    with tc.high_priority(offset=-100):
                        for i in range(IB):
                            src = st[f"psC1_{phi}"][:, i, 0:NPIX].rearrange(
                                "p (g q) -> p g q", g=NG, q=G_PIX)
                            relu_copy(ENG[(k + i) % 2], c1sb[:, :, i, :],
                                      src, bc1_s[:, phi:phi + 1])
                        nc.sync.dma_start(
                            out=xsp[phi][:, :, g0:g0 + IB, :],
                            in_=c1sb[:])

                NGRP = BL // IB if "A" in phases else 0
                if NGRP:
                    dma_head(0)
                    head_e0(0)
                    head_e1(0)
                for g in range(NGRP):
                    nxt = g + 1 if g + 1 < NGRP else None
                    if nxt is not None:
                        with tc.high_priority():
                            dma_head(nxt)
                    # body: c0P1, c0P3, c1P1, c0P2 (+ relus); spill copies
                    # (c1_out) are demoted below the next group's head
                    # copies so the chain-critical embed junction never
                    # queues behind them.
                    ps0 = c0_mm(g, 0)
                    c0_relu(g, 0, ps0, 0)
                    ps2 = c0_mm(g, 2)
                    c0_relu(g, 2, ps2, 1)
                    c1_mm(g, 0)
                    ps1 = c0_mm(g, 1)
                    c0_relu(g, 1, ps1, 2)
                    if nxt is not None:
                        with tc.high_priority():
                            head_e0(nxt)
                    c1_out(g, 0, 2)
                    c1_mm(g, 2)
                    if nxt is not None:
                        with tc.high_priority():
                            head_e1(nxt)
                    c1_out(g, 2, 0)
                    c1_mm(g, 1)
                    c1_out(g, 1, 1)
                    stash.pop(g, None)

            # ============================================================
            # phase B: 51200->512 linears (+ transpose to feat-major fp16)
            # phase C interleaved: quantize after phi=1, fc stacks after
            # ============================================================
            xT = {}      # feat-major [128, 4, BL] fp16 phi outputs (+bias)

            with (
                tc.tile_pool(name="pb", bufs=2) as pb,
                tc.tile_pool(name="pbps", bufs=2, space="PSUM") as pbps,
                tc.tile_pool(name="pc", bufs=1) as pc,
                tc.tile_pool(name="pcps", bufs=4, space="PSUM") as pcps,
            ):
                def linear_phi(phi):
                    wdt = FP8 if phi in W8 else BF16
                    acc = pbps.tile([BL, 512], F32, tag="acc")
                    for g in range(NG):
                        xg = pb.tile([128, BL, G_PIX], xdt[phi],
                                     bufs=3,
                                     tag=f"xg{xdt[phi]}")
                        nc.sync.dma_start(out=xg[:], in_=xsp[phi][:, g])
                        for t in range(G_PIX // W_PIX):
                            wt = pb.tile([128, W_PIX, 512], wdt,
                                         bufs=4,
                                         tag=f"wt{wdt}")
                            _eng = (nc.sync, nc.scalar, nc.gpsimd)[t % 3]
                            _eng.dma_start(
                                out=wt[:],
                                in_=wl[phi][g * (G_PIX // W_PIX) + t])
                            for j in range(W_PIX):
                                gp = g * G_PIX + t * W_PIX + j
                                nc.tensor.matmul(
                                    acc[:], xg[:, :, t * W_PIX + j],
                                    wt[:, j, :],
                                    start=(gp == 0), stop=(gp == NPIX - 1))
                    # PSUM [BL, 512] -> SBUF, transpose to [128, 4, BL]
                    # + bias (+ fp8 descale)
                    asb = pc.tile([BL, 512], F32, tag=f"asb{phi}")
                    nc.scalar.copy(asb[:], acc[:])
                    t = inter.tile([128, 4, BL], BF16, tag=f"xT{phi}")
                    sphi = 1.0 / F8_SCALE if phi in W8 else 1.0
                    for k in range(4):
                        pt = pcps.tile([128, 512], F32, tag="ps")
                        nc.tensor.transpose(pt[:, 0:BL],
                                            asb[:, k * 128:(k + 1) * 128],
                                            ident[0:BL, 0:BL])
                        nc.scalar.activation(t[:, k, :], pt[:, 0:BL],
                                             AF.Identity,
                                             bias=bl_s[:, phi, k:k + 1],
                                             scale=sphi)
                    xT[phi] = t

                def fc_layer(h_in, wname, relu, tag):
                    h_out = pc.tile([128, 4, BL], BF16, tag=tag)
                    for j in range(4):
                        ps = pcps.tile([128, 512], F32, tag="ps")
                        for k in range(4):
                            nc.tensor.matmul(
                                ps[:, 0:BL],
                                fcT_s[wname][:, k, j * 128:(j + 1) * 128],
                                h_in[:, k, :],
                                start=(k == 0), stop=(k == 3))
                        if relu:
                            nc.vector.tensor_scalar(
                                h_out[:, j, :], ps[:, 0:BL],
                                fcB_s[wname][:, j:j + 1], 0.0, ALU.add,
                                ALU.max)
                        else:
                            nc.vector.tensor_scalar(
                                h_out[:, j, :], ps[:, 0:BL],
                                fcB_s[wname][:, j:j + 1], None, ALU.add)
                    return h_out

                def normalize(h_in, with_escale, tag):
                    # returns h_in * 1/(||h||+eps) [* exp(scale)]
                    sq = pc.tile([128, 4, BL], F32, tag=tag + "_sq")
                    nc.vector.tensor_tensor(sq[:], h_in[:], h_in[:],
                                            ALU.mult)
                    pn = pcps.tile([128, 512], F32, tag="ps")
                    for k in range(4):
                        nc.tensor.matmul(pn[0:1, 0:BL], ones_col[:],
                                         sq[:, k, :],
                                         start=(k == 0), stop=(k == 3))
                    tn = pc.tile([1, BL], F32, tag=tag + "_tn")
                    nc.scalar.activation(tn[:], pn[0:1, 0:BL], AF.Sqrt)
                    nc.vector.tensor_scalar_add(tn[:], tn[:], EPS)
                    rn = pc.tile([1, BL], F32, tag=tag + "_rn")
                    nc.vector.reciprocal(rn[:], tn[:])
                    if with_escale:
                        nc.vector.tensor_scalar_mul(rn[:], rn[:], esc_s[:])
                    pbx = pcps.tile([128, 512], F32, tag="ps")
                    nc.tensor.matmul(pbx[:, 0:BL], ones_row[:], rn[:],
                                     start=True, stop=True)
                    h_out = pc.tile([128, 4, BL], BF16, tag=tag)
                    for k in range(4):
                        nc.vector.tensor_tensor(h_out[:, k, :],
                                                h_in[:, k, :],
                                                pbx[:, 0:BL], ALU.mult)
                    return h_out

                def _dummy_out():
                    dummy = pc.tile([BL, N_CORES * BL], F32, tag="dummy")
                    nc.vector.memset(dummy[:], 0.0)
                    nc.sync.dma_start(out=out_d[:], in_=dummy[:])

                def _bc():
                    if "B" not in phases:
                        _dummy_out()
                        return
                    # ---- diff first (feeds the longest chain: quantize)
                    linear_phi(1)

                    if "C" not in phases:
                        linear_phi(0)
                        linear_phi(2)
                        _dummy_out()
                        return

                    # quantize: scoreT[j,b] = zsq_j - 2 * (z @ diff)[j,b]
                    pG = pcps.tile([128, 512], F32, tag="ps")
                    for k in range(4):
                        nc.tensor.matmul(pG[0:64, 0:BL], zT_s[:, k, :],
                                         xT[1][:, k, :],
                                         start=(k == 0), stop=(k == 3))
                    scT = pc.tile([64, BL], F32, tag="scT")
                    nc.scalar.activation(scT[:], pG[0:64, 0:BL], AF.Identity,
                                         bias=zsq_s[:], scale=-2.0)
                    pSc = pcps.tile([128, 512], F32, tag="ps")
                    nc.tensor.transpose(pSc[0:BL, 0:64], scT[:],
                                        ident[0:64, 0:64])
                    scB = pc.tile([BL, 64], F32, tag="scB")
                    nc.vector.tensor_copy(scB[:], pSc[0:BL, 0:64])
                    mx = pc.tile([BL, 1], F32, tag="mx")
                    nc.vector.tensor_reduce(mx[:], scB[:],
                                            mybir.AxisListType.X, ALU.max)
                    ohB = pc.tile([BL, 64], F32, tag="ohB")
                    nc.vector.tensor_scalar(ohB[:], scB[:], mx[:], None,
                                            ALU.is_ge)
                    pOh = pcps.tile([128, 512], F32, tag="ps")
                    nc.tensor.transpose(pOh[0:64, 0:BL], ohB[:],
                                        ident[0:BL, 0:BL])
                    ohT = pc.tile([64, BL], BF16, tag="ohT")
                    nc.vector.tensor_copy(ohT[:], pOh[0:64, 0:BL])

                    # ---- sp path linear + fc stack, then kick the
                    # allgather early so it overlaps the p1 linear stream
                    linear_phi(2)
                    g1 = fc_layer(xT[2], "p3f0", True, "g1")
                    g2 = fc_layer(g1, "p3f1", False, "g2")
                    sp_outT = normalize(g2, False, "spoT")
                    nc.sync.dma_start(out=spT_d[:], in_=sp_outT[:])
                    if sim1:
                        for c in range(N_CORES):
                            nc.sync.dma_start(out=spall[c], in_=spT_d[:])
                    else:
                        nc.gpsimd.collective_compute(
                            "AllGather", ALU.bypass,
                            replica_groups=[list(range(N_CORES))],
                            ins=[spT_d[:]], outs=[spall[:]])

                    # ---- s path linear
                    linear_phi(0)

                    # z_matrix gather + add s_int
                    h0 = pc.tile([128, 4, BL], BF16, tag="h0")
                    for k in range(4):
                        pz = pcps.tile([128, 512], F32, tag="ps")
                        nc.tensor.matmul(pz[:, 0:BL],
                                         zrow_s[:, k * 128:(k + 1) * 128],
                                         ohT[:], start=True, stop=True)
                        nc.vector.scalar_tensor_tensor(
                            h0[:, k, :], pz[:, 0:BL], 0.0, xT[0][:, k, :],
                            ALU.bypass, ALU.add)

                    h1 = fc_layer(h0, "fc0", True, "h1")
                    h2 = fc_layer(h1, "fc1", False, "h2")
                    s_outT = normalize(h2, True, "soT")

                    # ---- gram block: out[my_b, all_b]
                    outsb = pc.tile([BL, N_CORES * BL], F32, tag="outsb")
                    spv = spall[:].rearrange("c p k b -> p k c b")
                    for h in range(n_chunks):
                        ncol = cpc * BL
                        pi = pcps.tile([128, 512], F32, tag="ps")
                        for k in range(4):
                            sps = pb.tile([128, cpc, BL], BF16, tag="sps")
                            nc.sync.dma_start(
                                out=sps[:],
                                in_=spv[:, k, h * cpc:(h + 1) * cpc, :])
                            spsf = sps[:].rearrange("p c b -> p (c b)")
                            nc.tensor.matmul(pi[0:BL, 0:ncol],
                                             s_outT[:, k, :], spsf,
                                             start=(k == 0), stop=(k == 3))
                        nc.scalar.copy(outsb[:, h * ncol:(h + 1) * ncol],
                                       pi[0:BL, 0:ncol])
                    nc.sync.dma_start(out=out_d[:], in_=outsb[:])

                _bc()

    nc.finalize()
    return nc


# --------------------------------------------------------------------------
# host-side input preparation
# --------------------------------------------------------------------------

def _embplanes(idx, tbl):
    """[n,20,20] int -> [n,72,484] f16: 9-tap-shifted embedding planes."""
    n = idx.shape[0]
    et = tbl[idx]                                    # [n,20,20,8]
    et = np.ascontiguousarray(et.transpose(0, 3, 1, 2))  # [n,8,20,20]
    out = np.zeros((n, 9, 8, PW, PW), BF_NP)
    for ky in range(3):
        for kx in range(3):
            py0, py1 = max(1, 2 - ky), min(21, 22 - ky)
            px0, px1 = max(1, 2 - kx), min(21, 22 - kx)
            y0, x0 = py0 + ky - 2, px0 + kx - 2
            out[:, ky * 3 + kx, :, py0:py1, px0:px1] = \
                et[:, :, y0:y0 + py1 - py0, x0:x0 + px1 - px0]
    return out.reshape(n, 72, NPAD)


def prep_shared(inputs):
    f = np.float32
    t = {}
    emb = np.asarray(inputs["emb_table"], f)
    norms = np.linalg.norm(emb, axis=1, keepdims=True)
    tbl = emb * np.where(norms > 1.0, f(1.0) / (norms + f(1e-7)), f(1.0))

    # conv0-e weights: wE0[(ky*3+kx)*8+e, o] = ec0_w[o,e,ky,kx]
    e0 = np.asarray(inputs["ec0_w"], f)                   # [32,8,3,3]
    t["wE0"] = np.ascontiguousarray(
        e0.transpose(2, 3, 1, 0).reshape(72, 32))

    e1 = np.asarray(inputs["ec1_w"], f)                   # [64,32,3,3]
    wE1 = np.zeros((96, 3, 64), f)
    for ky in range(3):
        for kx in range(3):
            wE1[ky * 32:(ky + 1) * 32, kx, :] = e1[:, :, ky, kx].T
    t["wE1"] = wE1

    wA0 = np.zeros((128, 9, 64), f)
    wB0 = np.zeros((128, 9, 64), f)
    wA1 = np.zeros((128, 9, 128), f)
    wB1 = np.zeros((128, 9, 128), f)
    for phi, p in enumerate(("p1", "p2", "p3")):
        c0 = np.asarray(inputs[p + "c0_w"], f)            # [64,64,3,3]
        c1 = np.asarray(inputs[p + "c1_w"], f)            # [128,64,3,3]
        for kx in range(3):
            wA0[0:64, phi * 3 + kx, :] = c0[:, :, 0, kx].T
            wA0[64:128, phi * 3 + kx, :] = c0[:, :, 1, kx].T
            wB0[64:128, phi * 3 + kx, :] = c0[:, :, 2, kx].T
            wA1[0:64, phi * 3 + kx, :] = c1[:, :, 0, kx].T
            wA1[64:128, phi * 3 + kx, :] = c1[:, :, 1, kx].T
            wB1[64:128, phi * 3 + kx, :] = c1[:, :, 2, kx].T
    t["wA0"], t["wB0"], t["wA1"], t["wB1"] = wA0, wB0, wA1, wB1

    t["b0"] = np.asarray(inputs["ec0_b"], f).reshape(32, 1)
    t["b1"] = np.asarray(inputs["ec1_b"], f).reshape(64, 1)
    t["bc0"] = np.stack([np.asarray(inputs[p + "c0_b"], f)
                         for p in ("p1", "p2", "p3")], axis=1)   # [64,3]
    t["bc1"] = np.stack([np.asarray(inputs[p + "c1_b"], f)
                         for p in ("p1", "p2", "p3")], axis=1)   # [128,3]

    # big linear weights: [NWT, 128, W_PIX, 512]; fp8 phis prescaled
    NWT = NPIX // W_PIX
    for phi, p in enumerate(("p1", "p2", "p3")):
        w = np.asarray(inputs[p + "l_w"], f).reshape(512, 128, NWT, W_PIX)
        w = np.ascontiguousarray(w.transpose(2, 1, 3, 0))
        if phi in W8:
            t[f"wl{phi}"] = (w * F8_SCALE).astype(F8_NP)
        else:
            t[f"wl{phi}"] = w.astype(BF_NP)
    t["blpk"] = np.stack([np.asarray(inputs[p + "l_b"], f).reshape(4, 128).T
                          for p in ("p1", "p2", "p3")], axis=1)  # [128,3,4]

    for n in ("fc0", "fc1", "p3f0", "p3f1"):
        w = np.asarray(inputs[n + "_w"], f)               # [512,512] out,in
        t[n + "T"] = np.ascontiguousarray(
            w.T.reshape(4, 128, 512).transpose(1, 0, 2)).astype(BF_NP)
        t[n + "b"] = np.asarray(inputs[n + "_b"], f).reshape(4, 128).T

    z = np.asarray(inputs["z_vec"], f)                    # [64,512]
    t["zT"] = np.ascontiguousarray(
        z.T.reshape(4, 128, 64).transpose(1, 0, 2)).astype(BF_NP)
    t["zrow"] = z.astype(BF_NP)
    t["zsq"] = (z.astype(f) ** 2).sum(axis=1, keepdims=True).astype(f)
    t["esc"] = np.exp(np.asarray(inputs["scale"], f)).reshape(1, 1)
    for k in ("wE0", "wE1", "wA0", "wB0", "wA1", "wB1"):
        t[k] = t[k].astype(BF_NP)
    t["_tbl"] = tbl
    return t


def make_in_maps(inputs, BL):
    shared = prep_shared(inputs)
    tbl = shared.pop("_tbl")
    s = np.asarray(inputs["s"])
    sp = np.asarray(inputs["s_prime"])
    maps = []
    for c in range(N_CORES):
        m = dict(shared)
        m["embp_s"] = _embplanes(s[c * BL:(c + 1) * BL], tbl)
        m["embp_sp"] = _embplanes(sp[c * BL:(c + 1) * BL], tbl)
        maps.append(m)
    return maps


# --------------------------------------------------------------------------
# cached runner (jit once, reuse across calls)
# --------------------------------------------------------------------------

class Runner:
    def __init__(self, BL, nc=None):
        import jax
        from jax.sharding import Mesh, PartitionSpec, NamedSharding
        from jax.experimental.shard_map import shard_map
        from concourse import bass2jax
        self.BL = BL
        self.nc = nc if nc is not None else build_program(BL)
        nc = self.nc
        bass2jax.install_neuronx_cc_hook()

        partition_name = (nc.partition_id_tensor.name
                          if nc.partition_id_tensor else None)
        in_names, out_names, out_avals, zero_outs = [], [], [], []
        for alloc in nc.m.functions[0].allocations:
            if not isinstance(alloc, mybir.MemoryLocationSet):
                continue
            name = alloc.memorylocations[0].name
            if alloc.kind == "ExternalInput":
                if name != partition_name:
                    in_names.append(name)
            elif alloc.kind == "ExternalOutput":
                shape = tuple(alloc.tensor_shape)
                dtype = mybir.dt.np(alloc.dtype)
                out_names.append(name)
                out_avals.append(jax.core.ShapedArray(shape, dtype))
                zero_outs.append(np.zeros(shape, dtype))
        self.in_names, self.out_names = list(in_names), out_names
        self.out_avals, self.zero_outs = out_avals, zero_outs
        n_params, n_outs = len(in_names), len(out_avals)
        all_in_names = in_names + out_names
        if partition_name is not None:
            all_in_names = all_in_names + [partition_name]

        def _body(*args):
            operands = list(args)
            if partition_name is not None:
                operands.append(bass2jax.partition_id_tensor())
            return tuple(bass2jax._bass_exec_p.bind(
                *operands,
                out_avals=tuple(out_avals),
                in_names=tuple(all_in_names),
                out_names=tuple(out_names),
                lowering_input_output_aliases=(),
                sim_require_finite=True,
                sim_require_nnan=True,
                nc=nc,
            ))

        devices = jax.devices()[:N_CORES]
        self.mesh = Mesh(np.asarray(devices), ("core",))
        in_specs = (PartitionSpec("core"),) * (n_params + n_outs)
        out_specs = (PartitionSpec("core"),) * n_outs
        self.sharding = NamedSharding(self.mesh, PartitionSpec("core"))
        self.jitted = jax.jit(
            shard_map(_body, mesh=self.mesh, in_specs=in_specs,
                      out_specs=out_specs, check_rep=False),
            donate_argnums=tuple(range(n_params, n_params + n_outs)),
            keep_unused=True)
        self._staged = None
        self._jax = jax

    def stage(self, in_maps):
        """device_put the concatenated inputs once."""
        jax = self._jax
        concat = [np.concatenate([np.asarray(m[n]) for m in in_maps], axis=0)
                  for n in self.in_names]
        self._staged = [jax.device_put(a, self.sharding) for a in concat]
        jax.block_until_ready(self._staged)

    def run(self):
        jax = self._jax
        zo = [jax.device_put(
            np.zeros((N_CORES * z.shape[0], *z.shape[1:]), z.dtype),
            self.sharding) for z in self.zero_outs]
        jax.block_until_ready(zo)
        outs = self.jitted(*self._staged, *zo)
        jax.block_until_ready(outs)
        return outs

    def output(self, outs):
        o = np.asarray(outs[self.out_names.index("out")])
        return o  # [N_CORES*BL, N_CORES*BL]


_RUNNER_CACHE = {}


def _get_runner(BL):
    if BL not in _RUNNER_CACHE:
        _RUNNER_CACHE[BL] = Runner(BL)
    return _RUNNER_CACHE[BL]


def kernel(**inputs):
    assert int(np.asarray(inputs["downscale_factor"])) == 1
    BL = np.asarray(inputs["s"]).shape[0] // N_CORES
    r = _get_runner(BL)
    r.stage(make_in_maps(inputs, BL))
    outs = r.run()
    return r.output(outs).astype(np.float32)



# revision 16
# speedup vs baseline: 1.4686x; 1.4686x over previous
"""Trainium2 Bass kernel for nn_DSSMReverse (DSSM embed/conv/VQ/Gram model).

Strategy: data-parallel over batch across 8 NeuronCores (128 images each).
 - Embedding+conv0 are fully composed on host: 9-tap-shifted embedding
   planes (72 = 8ch x 9 taps) make conv0 a single K=72 matmul per image.
 - Remaining 3x3 convs run as shifted-window matmuls with row-group
   packing (K=96/128+64); relu/shift copies are spread across the
   ACT/DVE/Pool engines; emission is stage-major with phi staggering so
   the PE rarely waits on the relu copies.
 - phi conv outputs [128c x 400px] spill to DRAM in pixel-group-major
   layout; the 51200->512 linears stream x tiles [128,128,100] and
   weight tiles [128,10,512] (both contiguous) from HBM. Weights can be
   fp8-e4m3 per phi (scale 128, descaled in the bias epilogue).
 - Phase C (feat-major [4x128, B], fp16): VQ codebook argmax+gather via
   matmuls + PE transposes, fc stacks, L2 normalize via ones-matmul,
   AllGather of sp_out, per-core [128, 1024] block of the Gram matrix.
"""

import os
import numpy as np
import ml_dtypes

BF_NP = np.float16
F8_NP = ml_dtypes.float8_e4m3

import concourse.bacc as bacc
import concourse.bass as bass
import concourse.mybir as mybir
import concourse.tile as tile
from concourse.masks import make_identity

N_CORES = 8
B_FULL = 1024
H = W = 20
PW = 22            # padded plane width
NPIX = 400
NPAD = 484         # 22*22
EPS = 1e-4
F32 = mybir.dt.float32
BF16 = mybir.dt.float16    # 16-bit matmul dtype (fp16: 1 cyc/row)
FP8 = mybir.dt.float8e4
AF = mybir.ActivationFunctionType
ALU = mybir.AluOpType

W8 = (0,)          # phis whose big-linear weights are fp8 (0=p1,1=p2,2=p3)
                   # p1 is safe (2e-3, VQ-diluted); p2 flips VQ indices
                   # (catastrophic); p3 costs ~1.6e-2 (thin margin)
X8 = ()            # conv features stay fp16: fp8-x only pays off on a
                   # DMA-bound phi, and the only fp8-safe phi (p1) is
                   # already PE-bound
F8_SCALE = 128.0
W_PIX = 10         # pixels per streamed weight tile in phase B
G_PIX = 100        # pixels per x tile in phase B


# --------------------------------------------------------------------------
# device program
# --------------------------------------------------------------------------

_STAGE_LOG = []            # (label, first-instruction-name) when tracing


def build_program(BL, phases="ABC", ib=2, sim1=False, trace_stages=False):
    nc = bacc.Bacc("TRN2", target_bir_lowering=False, debug=False,
                   num_devices=1 if sim1 else N_CORES)
    _STAGE_LOG.clear()

    def mark(label):
        if trace_stages:
            _STAGE_LOG.append((label, nc.get_next_instruction_name()))

    def inp(name, shape, dt=F32):
        return nc.dram_tensor(name, shape, dt, kind="ExternalInput").ap()

    embp_s = inp("embp_s", [BL, 72, NPAD], BF16)
    embp_sp = inp("embp_sp", [BL, 72, NPAD], BF16)
    wE0 = inp("wE0", [72, 32], BF16)
    wE1 = inp("wE1", [96, 3, 64], BF16)
    wA0 = inp("wA0", [128, 9, 64], BF16)
    wB0 = inp("wB0", [128, 9, 64], BF16)
    wA1 = inp("wA1", [128, 9, 128], BF16)
    wB1 = inp("wB1", [128, 9, 128], BF16)
    wA0w = inp("wA0sw", [128, 9, 64], BF16)
    wB0w = inp("wB0sw", [128, 9, 64], BF16)
    wA1w = inp("wA1sw", [128, 9, 128], BF16)
    wB1w = inp("wB1sw", [128, 9, 128], BF16)
    b0 = inp("b0", [32, 1])
    b1 = inp("b1", [64, 1])
    bc0 = inp("bc0", [64, 3])
    bc1 = inp("bc1", [128, 3])
    NWT = NPIX // W_PIX
    wl = [inp(f"wl{p}", [NWT, 128, W_PIX, 512], FP8 if p in W8 else BF16)
          for p in range(3)]
    blpk = inp("blpk", [128, 3, 4])
    fcT = {n: inp(n + "T", [128, 4, 512], BF16)
           for n in ("fc0", "fc1", "p3f0", "p3f1")}
    fcB = {n: inp(n + "b", [128, 4]) for n in ("fc0", "fc1", "p3f0", "p3f1")}
    zT = inp("zT", [128, 4, 64], BF16)
    zrow = inp("zrow", [64, 512], BF16)
    zsq = inp("zsq", [64, 1])
    esc = inp("esc", [1, 1])

    out_d = nc.dram_tensor("out", [BL, N_CORES * BL], F32,
                           kind="ExternalOutput").ap()

    cpc = min(N_CORES, 512 // BL)          # gram col-cores per matmul chunk
    n_chunks = (N_CORES + cpc - 1) // cpc
    NG = NPIX // G_PIX                     # x pixel groups in phase B

    with tile.TileContext(nc) as tc:
        with (
            tc.tile_pool(name="const", bufs=1) as cst,
            tc.tile_pool(name="inter", bufs=1) as inter,
            tc.tile_pool(name="dram", bufs=1, space="DRAM") as dram,
        ):
            # ---- persistent weights -> SBUF
            def load(ap, shape, tag, dt=F32):
                t = cst.tile(shape, dt, tag=tag, name=tag)
                nc.sync.dma_start(out=t[:], in_=ap[:])
                return t

            wE0_s = load(wE0, [72, 32], "wE0", BF16)
            wE1_s = load(wE1, [96, 3, 64], "wE1", BF16)
            wA0_s = load(wA0, [128, 9, 64], "wA0", BF16)
            wB0_s = load(wB0, [128, 9, 64], "wB0", BF16)
            wA1_s = load(wA1, [128, 9, 128], "wA1", BF16)
            wB1_s = load(wB1, [128, 9, 128], "wB1", BF16)
            wA0w_s = load(wA0w, [128, 9, 64], "wA0sw", BF16)
            wB0w_s = load(wB0w, [128, 9, 64], "wB0sw", BF16)
            wA1w_s = load(wA1w, [128, 9, 128], "wA1sw", BF16)
            wB1w_s = load(wB1w, [128, 9, 128], "wB1sw", BF16)
            b0_s = load(b0, [32, 1], "b0")
            b1_s = load(b1, [64, 1], "b1")
            bc0_s = load(bc0, [64, 3], "bc0")
            bc1_s = load(bc1, [128, 3], "bc1")
            bl_s = load(blpk, [128, 3, 4], "blpk")
            # phase-C-only weights: demote so they don't block the first
            # embed-plane DMAs on the sync queue at kernel start
            with tc.high_priority(offset=-100000):
                fcT_s = {n: load(fcT[n], [128, 4, 512], n + "T", BF16)
                         for n in fcT}
                fcB_s = {n: load(fcB[n], [128, 4], n + "b") for n in fcB}
                zT_s = load(zT, [128, 4, 64], "zT", BF16)
                zrow_s = load(zrow, [64, 512], "zrow", BF16)
                zsq_s = load(zsq, [64, 1], "zsq")
                esc_s = load(esc, [1, 1], "esc")

            ident = cst.tile([128, 128], F32, tag="ident")
            make_identity(nc, ident[:])
            ones_col = cst.tile([128, 1], F32, tag="ones_col")
            nc.vector.memset(ones_col[:], 1.0)
            ones_row = cst.tile([1, 128], F32, tag="ones_row")
            nc.vector.memset(ones_row[:], 1.0)

            # DRAM spill for phi conv1 outputs, [c, pxgroup, img, px_in_g]
            xdt = [FP8 if i in X8 else BF16 for i in range(3)]
            xsp = [dram.tile([128, NG, BL, G_PIX], xdt[i], tag=f"xsp{i}",
                             name=f"xsp{i}") for i in range(3)]
            # collective buffers
            spT_d = dram.tile([128, 4, BL], BF16, tag="spT")
            spall = dram.tile([N_CORES, 128, 4, BL], BF16, tag="spall",
                              **({} if sim1 else {"addr_space": "Shared"}))

            # ============================================================
            # phase A: conv stacks, IB images per instruction group (fp16)
            # ============================================================
            IB = ib if BL % ib == 0 else 1
            with (
                tc.tile_pool(name="pa", bufs=2) as pa,
                tc.tile_pool(name="papsh", bufs=2, space="PSUM") as papsh,
                tc.tile_pool(name="papsb", bufs=2, space="PSUM") as papsb,
            ):
                def win(t4, i, y0, y1, kx):
                    """conv window of image i: [P, 20, 20] at row y0..y1-1."""
                    v = t4[:, i, :].rearrange("p (y x) -> p y x", y=PW, x=PW)
                    return v[:, y0:y1, kx:kx + 20]

                # rotating engine choice for relu copies (Pool is ~1.3x
                # slower per element, so it gets a smaller share)
                ENG = (nc.scalar, nc.vector, nc.gpsimd)

                def relu_copy(eng, dst, src, bias):
                    if eng is nc.scalar:
                        eng.activation(dst, src, AF.Relu, bias=bias)
                    else:
                        eng.tensor_scalar(dst, src, bias, 0.0, ALU.add,
                                          ALU.max)

                def dup_copy(dst, src):
                    # Pool cannot read PSUM: shifted duplicates are plain
                    # SBUF->SBUF copies of already-relu'd data
                    nc.gpsimd.tensor_copy(dst, src)

                stash = {}   # group idx -> dict of live tiles

                def dma_head(g):
                    mark(f'dma_head:{g}')
                    st = stash.setdefault(g, {})
                    g0 = g * IB
                    for m, ap in (("s", embp_s), ("sp", embp_sp)):
                        t = pa.tile([72, IB, NPAD], BF16, tag=f"embp_{m}",
                                    bufs=3)
                        nc.sync.dma_start(
                            out=t[:],
                            in_=ap[g0:g0 + IB].rearrange("i p n -> p i n"))
                        st[f"embp_{m}"] = t

                def head_e0(g):
                    # E0 matmuls (K=72, 9 taps host-composed): 4-way
                    # col-tiled concurrent pack [s_i0|sp_i0|s_i1|sp_i1],
                    # then relu + 2 flat row-shift DMA dups per tensor
                    mark(f'head_e0:{g}')
                    st = stash[g]
                    g0 = g * IB
                    ps = papsh.tile([128, IB, 512], F32, tag="ps")
                    for i in range(IB):
                        for mi, m in enumerate(("s", "sp")):
                            q = mi + 2 * i
                            v = st[f"embp_{m}"][:, i, :].rearrange(
                                "p (y x) -> p y x", y=PW, x=PW)
                            nc.tensor.matmul(
                                ps[q * 32:(q + 1) * 32, 0, 0:NPIX],
                                wE0_s[:], v[:, 1:21, 1:21],
                                start=True, stop=True,
                                tile_position=(0, q * 32))
                    for mi, m in enumerate(("s", "sp")):
                        eb = pa.tile([96, IB, NPAD], BF16, tag="e0big",
                                     bufs=3)
                        if g0 < 3 * IB:
                            nc.gpsimd.memset(eb[:], 0.0)
                        ev = eb[:].rearrange("p i (y x) -> p i y x",
                                             y=PW, x=PW)
                        for i in range(IB):
                            q = mi + 2 * i
                            pv = ps[q * 32:(q + 1) * 32, 0,
                                    0:NPIX].rearrange(
                                "p (y x) -> p y x", y=20, x=20)
                            relu_copy((nc.scalar, nc.vector)[i % 2],
                                      ev[0:32, i, 1:21, 1:21], pv[:],
                                      b0_s[:])
                        # row-shifted duplicates: flat contiguous DMA
                        # (pad cols/rows carry zeros along correctly)
                        nc.gpsimd.dma_start(
                            out=eb[32:64, :, 0:462],
                            in_=eb[0:32, :, 22:484])
                        nc.gpsimd.dma_start(
                            out=eb[64:96, :, 0:440],
                            in_=eb[0:32, :, 44:484])
                        st[f"e0big_{m}"] = eb

                def head_e1(g):
                    # E1 matmuls (K=96, 3 kx taps) both tensors into one
                    # psum [128,...] + embbig relus + diff
                    mark(f'head_e1:{g}')
                    st = stash[g]
                    g0 = g * IB
                    ps = papsh.tile([128, IB, 512], F32, tag="ps")
                    for mi, m in enumerate(("s", "sp")):
                        for kx in range(3):
                            for i in range(IB):
                                nc.tensor.matmul(
                                    ps[mi * 64:(mi + 1) * 64, i, 0:NPIX],
                                    wE1_s[:, kx, :],
                                    win(st[f"e0big_{m}"], i, 0, 20, kx),
                                    start=(kx == 0), stop=(kx == 2))
                    for mi, m in enumerate(("s", "sp")):
                        eb = pa.tile([128, IB, NPAD], BF16, tag="emb",
                                     bufs=4)
                        if g0 < 4 * IB:
                            nc.gpsimd.memset(eb[:], 0.0)
                        ebv = eb[:].rearrange("p i (y x) -> p i y x",
                                              y=PW, x=PW)
                        for i in range(IB):
                            lo = 0 if i % 2 == 0 else 64
                            p1v = ps[mi * 64:(mi + 1) * 64, i,
                                     0:NPIX].rearrange(
                                "p (y x) -> p y x", y=20, x=20)
                            relu_copy(nc.scalar if (mi + i) % 2 == 0
                                      else nc.vector,
                                      ebv[lo:lo + 64, i, 1:21, 1:21],
                                      p1v[:], b1_s[:])
                        for i in range(IB):
                            if i % 2 == 0:
                                nc.gpsimd.dma_start(
                                    out=eb[64:128, i, 0:462],
                                    in_=eb[0:64, i, 22:484])
                            else:
                                nc.gpsimd.dma_start(
                                    out=eb[0:64, i, 0:462],
                                    in_=eb[64:128, i, 22:484])
                        st[f"emb_{m}"] = eb
                    diff = pa.tile([128, IB, NPAD], BF16, tag="diff",
                                   bufs=2)
                    nc.gpsimd.tensor_tensor(diff[:], st["emb_sp"][:],
                                            st["emb_s"][:], ALU.subtract)
                    st["diff"] = diff

                def c0_mm(g, phi):
                    # image-pair col-packed: i0 -> psum 0:64, i1 -> 64:128;
                    # A pairs at col 0/64, B pairs at (64, 0/64)
                    mark(f'c0_mm:{g}:{phi}')
                    st = stash[g]
                    src = {0: st["emb_s"], 1: st["diff"],
                           2: st["emb_sp"]}[phi]
                    ps = papsb.tile([128, IB // 2, 512], F32, tag="ps")
                    for kx in range(3):
                        for i in range(IB):
                            nc.tensor.matmul(
                                ps[64 * (i % 2):64 * (i % 2) + 64,
                                   i // 2, 0:NPIX],
                                wA0_s[:, phi * 3 + kx, :],
                                win(src, i, 0, 20, kx),
                                start=(kx == 0), stop=False,
                                tile_position=(0, 64 * (i % 2)))
                    for kx in range(3):
                        for i in range(IB):
                            nc.tensor.matmul(
                                ps[64 * (i % 2):64 * (i % 2) + 64,
                                   i // 2, 0:NPIX],
                                wB0_s[64:128, phi * 3 + kx, :],
                                win(src, i, 1, 21, kx)[64:128],
                                start=False, stop=(kx == 2),
                                tile_position=(64, 64 * (i % 2)))
                    return ps

                def c0_relu(g, phi, ps, k):
                    mark(f'c0_relu:{g}:{phi}')
                    st = stash[g]
                    cb = pa.tile([128, IB, NPAD], BF16, tag="c0big",
                                 bufs=3)
                    if g * IB < 3 * IB:
                        nc.gpsimd.memset(cb[:], 0.0)
                    cv = cb[:].rearrange("p i (y x) -> p i y x", y=PW, x=PW)
                    for i in range(IB):
                        c0v = ps[64 * (i % 2):64 * (i % 2) + 64, i // 2,
                                 0:NPIX].rearrange(
                            "p (y x) -> p y x", y=20, x=20)
                        relu_copy(ENG[(k + i) % 2], cv[0:64, i, 1:21, 1:21],
                                  c0v[:], bc0_s[:, phi:phi + 1])
                    nc.gpsimd.dma_start(
                        out=cb[64:128, :, 0:462],
                        in_=cb[0:64, :, 22:484])
                    st[f"c0big{phi}"] = cb

                def c1_mm(g, phi):
                    mark(f'c1_mm:{g}:{phi}')
                    st = stash[g]
                    src = st[f"c0big{phi}"]
                    ps = papsb.tile([128, IB, 512], F32, tag="ps")
                    for i in range(IB):
                        for kx in range(3):
                            nc.tensor.matmul(
                                ps[:, i, 0:NPIX],
                                wA1_s[:, phi * 3 + kx, :],
                                win(src, i, 0, 20, kx),
                                start=(kx == 0), stop=False)
                        for kx in range(3):
                            nc.tensor.matmul(
                                ps[:, i, 0:NPIX],
                                wB1_s[64:128, phi * 3 + kx, :],
                                win(src, i, 1, 21, kx)[64:128],
                                start=False, stop=(kx == 2))
                    st[f"psC1_{phi}"] = ps

                def c1_out(g, phi, k):
                    mark(f'c1_out:{g}:{phi}')
                    st = stash[g]
                    g0 = g * IB
                    c1sb = pa.tile([128, NG, IB, G_PIX], xdt[phi],
                                   tag=f"c1sb{xdt[phi]}", bufs=3)
                    # spill copies are off the critical path: demote them
                    # so chain-critical relu copies win engine races
                    with tc.high_priority(offset=-100):
                        for i in range(IB):
                            src = st[f"psC1_{phi}"][:, i, 0:NPIX].rearrange(
                                "p (g q) -> p g q", g=NG, q=G_PIX)
                            relu_copy(ENG[(k + i) % 2], c1sb[:, :, i, :],
                                      src, bc1_s[:, phi:phi + 1])
                        nc.sync.dma_start(
                            out=xsp[phi][:, :, g0:g0 + IB, :],
                            in_=c1sb[:])

                NGRP = BL // IB if "A" in phases else 0
                if NGRP:
                    dma_head(0)
                    head_e0(0)
                    head_e1(0)
                for g in range(NGRP):
                    nxt = g + 1 if g + 1 < NGRP else None
                    if nxt is not None:
                        with tc.high_priority():
                            dma_head(nxt)
                    # body: c0P1, c0P3, c1P1, c0P2 (+ relus); spill copies
                    # (c1_out) are demoted below the next group's head
                    # copies so the chain-critical embed junction never
                    # queues behind them.
                    ps0 = c0_mm(g, 0)
                    c0_relu(g, 0, ps0, 0)
                    ps2 = c0_mm(g, 2)
                    c0_relu(g, 2, ps2, 1)
                    c1_mm(g, 0)
                    ps1 = c0_mm(g, 1)
                    c0_relu(g, 1, ps1, 2)
                    if nxt is not None:
                        with tc.high_priority():
                            head_e0(nxt)
                    c1_out(g, 0, 2)
                    c1_mm(g, 2)
                    if nxt is not None:
                        with tc.high_priority():
                            head_e1(nxt)
                    c1_out(g, 2, 0)
                    c1_mm(g, 1)
                    c1_out(g, 1, 1)
                    stash.pop(g, None)

            # ============================================================
            # phase B: 51200->512 linears (+ transpose to feat-major fp16)
            # phase C interleaved: quantize after phi=1, fc stacks after
            # ============================================================
            xT = {}      # feat-major [128, 4, BL] fp16 phi outputs (+bias)

            with (
                tc.tile_pool(name="pb", bufs=2) as pb,
                tc.tile_pool(name="pbps", bufs=2, space="PSUM") as pbps,
                tc.tile_pool(name="pc", bufs=1) as pc,
                tc.tile_pool(name="pcps", bufs=4, space="PSUM") as pcps,
            ):
                def linear_phi(phi):
                    mark(f'linear_phi:{phi}')
                    wdt = FP8 if phi in W8 else BF16
                    acc = pbps.tile([BL, 512], F32, tag="acc")
                    for g in range(NG):
                        xg = pb.tile([128, BL, G_PIX], xdt[phi],
                                     bufs=3,
                                     tag=f"xg{xdt[phi]}")
                        nc.sync.dma_start(out=xg[:], in_=xsp[phi][:, g])
                        for t in range(G_PIX // W_PIX):
                            wt = pb.tile([128, W_PIX, 512], wdt,
                                         bufs=4,
                                         tag=f"wt{wdt}")
                            _eng = (nc.sync, nc.scalar, nc.gpsimd)[t % 3]
                            _eng.dma_start(
                                out=wt[:],
                                in_=wl[phi][g * (G_PIX // W_PIX) + t])
                            for j in range(W_PIX):
                                gp = g * G_PIX + t * W_PIX + j
                                nc.tensor.matmul(
                                    acc[:], xg[:, :, t * W_PIX + j],
                                    wt[:, j, :],
                                    start=(gp == 0), stop=(gp == NPIX - 1))
                    # PSUM [BL, 512] -> SBUF, transpose to [128, 4, BL]
                    # + bias (+ fp8 descale)
                    asb = pc.tile([BL, 512], F32, tag=f"asb{phi}")
                    nc.scalar.copy(asb[:], acc[:])
                    t = inter.tile([128, 4, BL], BF16, tag=f"xT{phi}")
                    sphi = 1.0 / F8_SCALE if phi in W8 else 1.0
                    for k in range(4):
                        pt = pcps.tile([128, 512], F32, tag="ps")
                        nc.tensor.transpose(pt[:, 0:BL],
                                            asb[:, k * 128:(k + 1) * 128],
                                            ident[0:BL, 0:BL])
                        nc.scalar.activation(t[:, k, :], pt[:, 0:BL],
                                             AF.Identity,
                                             bias=bl_s[:, phi, k:k + 1],
                                             scale=sphi)
                    xT[phi] = t

                def fc_layer(h_in, wname, relu, tag):
                    mark(f'fc_layer:{wname}')
                    h_out = pc.tile([128, 4, BL], BF16, tag=tag)
                    for j in range(4):
                        ps = pcps.tile([128, 512], F32, tag="ps")
                        for k in range(4):
                            nc.tensor.matmul(
                                ps[:, 0:BL],
                                fcT_s[wname][:, k, j * 128:(j + 1) * 128],
                                h_in[:, k, :],
                                start=(k == 0), stop=(k == 3))
                        if relu:
                            nc.vector.tensor_scalar(
                                h_out[:, j, :], ps[:, 0:BL],
                                fcB_s[wname][:, j:j + 1], 0.0, ALU.add,
                                ALU.max)
                        else:
                            nc.vector.tensor_scalar(
                                h_out[:, j, :], ps[:, 0:BL],
                                fcB_s[wname][:, j:j + 1], None, ALU.add)
                    return h_out

                def normalize(h_in, with_escale, tag):
                    mark(f'normalize:{tag}')
                    # returns h_in * 1/(||h||+eps) [* exp(scale)]
                    sq = pc.tile([128, 4, BL], F32, tag=tag + "_sq")
                    nc.vector.tensor_tensor(sq[:], h_in[:], h_in[:],
                                            ALU.mult)
                    pn = pcps.tile([128, 512], F32, tag="ps")
                    for k in range(4):
                        nc.tensor.matmul(pn[0:1, 0:BL], ones_col[:],
                                         sq[:, k, :],
                                         start=(k == 0), stop=(k == 3))
                    tn = pc.tile([1, BL], F32, tag=tag + "_tn")
                    nc.scalar.activation(tn[:], pn[0:1, 0:BL], AF.Sqrt)
                    nc.vector.tensor_scalar_add(tn[:], tn[:], EPS)
                    rn = pc.tile([1, BL], F32, tag=tag + "_rn")
                    nc.vector.reciprocal(rn[:], tn[:])
                    if with_escale:
                        nc.vector.tensor_scalar_mul(rn[:], rn[:], esc_s[:])
                    pbx = pcps.tile([128, 512], F32, tag="ps")
                    nc.tensor.matmul(pbx[:, 0:BL], ones_row[:], rn[:],
                                     start=True, stop=True)
                    h_out = pc.tile([128, 4, BL], BF16, tag=tag)
                    for k in range(4):
                        nc.vector.tensor_tensor(h_out[:, k, :],
                                                h_in[:, k, :],
                                                pbx[:, 0:BL], ALU.mult)
                    return h_out

                def _dummy_out():
                    dummy = pc.tile([BL, N_CORES * BL], F32, tag="dummy")
                    nc.vector.memset(dummy[:], 0.0)
                    nc.sync.dma_start(out=out_d[:], in_=dummy[:])

                def _bc():
                    if "B" not in phases:
                        _dummy_out()
                        return
                    # ---- sp path FIRST: its fc stack + normalize feed the
                    # AllGather, which then hides behind the p2/p1 streams
                    linear_phi(2)

                    if "C" not in phases:
                        linear_phi(1)
                        linear_phi(0)
                        _dummy_out()
                        return

                    g1 = fc_layer(xT[2], "p3f0", True, "g1")
                    g2 = fc_layer(g1, "p3f1", False, "g2")
                    sp_outT = normalize(g2, False, "spoT")
                    mark('allgather')
                    nc.sync.dma_start(out=spT_d[:], in_=sp_outT[:])
                    if sim1:
                        for c in range(N_CORES):
                            nc.sync.dma_start(out=spall[c], in_=spT_d[:])
                    else:
                        nc.gpsimd.collective_compute(
                            "AllGather", ALU.bypass,
                            replica_groups=[list(range(N_CORES))],
                            ins=[spT_d[:]], outs=[spall[:]])

                    # ---- diff path linear + quantize
                    linear_phi(1)

                    # quantize: scoreT[j,b] = zsq_j - 2 * (z @ diff)[j,b]
                    mark('quantize')
                    pG = pcps.tile([128, 512], F32, tag="ps")
                    for k in range(4):
                        nc.tensor.matmul(pG[0:64, 0:BL], zT_s[:, k, :],
                                         xT[1][:, k, :],
                                         start=(k == 0), stop=(k == 3))
                    scT = pc.tile([64, BL], F32, tag="scT")
                    nc.scalar.activation(scT[:], pG[0:64, 0:BL], AF.Identity,
                                         bias=zsq_s[:], scale=-2.0)
                    pSc = pcps.tile([128, 512], F32, tag="ps")
                    nc.tensor.transpose(pSc[0:BL, 0:64], scT[:],
                                        ident[0:64, 0:64])
                    scB = pc.tile([BL, 64], F32, tag="scB")
                    nc.vector.tensor_copy(scB[:], pSc[0:BL, 0:64])
                    mx = pc.tile([BL, 1], F32, tag="mx")
                    nc.vector.tensor_reduce(mx[:], scB[:],
                                            mybir.AxisListType.X, ALU.max)
                    ohB = pc.tile([BL, 64], F32, tag="ohB")
                    nc.vector.tensor_scalar(ohB[:], scB[:], mx[:], None,
                                            ALU.is_ge)
                    pOh = pcps.tile([128, 512], F32, tag="ps")
                    nc.tensor.transpose(pOh[0:64, 0:BL], ohB[:],
                                        ident[0:BL, 0:BL])
                    ohT = pc.tile([64, BL], BF16, tag="ohT")
                    nc.vector.tensor_copy(ohT[:], pOh[0:64, 0:BL])

                    # ---- s path linear
                    linear_phi(0)

                    # z_matrix gather + add s_int
                    mark('zgather')
                    h0 = pc.tile([128, 4, BL], BF16, tag="h0")
                    for k in range(4):
                        pz = pcps.tile([128, 512], F32, tag="ps")
                        nc.tensor.matmul(pz[:, 0:BL],
                                         zrow_s[:, k * 128:(k + 1) * 128],
                                         ohT[:], start=True, stop=True)
                        nc.vector.scalar_tensor_tensor(
                            h0[:, k, :], pz[:, 0:BL], 0.0, xT[0][:, k, :],
                            ALU.bypass, ALU.add)

                    h1 = fc_layer(h0, "fc0", True, "h1")
                    h2 = fc_layer(h1, "fc1", False, "h2")
                    s_outT = normalize(h2, True, "soT")

                    # ---- gram block: out[my_b, all_b]
                    mark('gram')
                    outsb = pc.tile([BL, N_CORES * BL], F32, tag="outsb")
                    spv = spall[:].rearrange("c p k b -> p k c b")
                    for h in range(n_chunks):
                        ncol = cpc * BL
                        pi = pcps.tile([128, 512], F32, tag="ps")
                        for k in range(4):
                            sps = pb.tile([128, cpc, BL], BF16, tag="sps")
                            nc.sync.dma_start(
                                out=sps[:],
                                in_=spv[:, k, h * cpc:(h + 1) * cpc, :])
                            spsf = sps[:].rearrange("p c b -> p (c b)")
                            nc.tensor.matmul(pi[0:BL, 0:ncol],
                                             s_outT[:, k, :], spsf,
                                             start=(k == 0), stop=(k == 3))
                        nc.scalar.copy(outsb[:, h * ncol:(h + 1) * ncol],
                                       pi[0:BL, 0:ncol])
                    nc.sync.dma_start(out=out_d[:], in_=outsb[:])

                _bc()

    nc.finalize()
    return nc


# --------------------------------------------------------------------------
# host-side input preparation
# --------------------------------------------------------------------------

def _embplanes(idx, tbl):
    """[n,20,20] int -> [n,72,484] f16: 9-tap-shifted embedding planes."""
    n = idx.shape[0]
    et = tbl[idx]                                    # [n,20,20,8]
    et = np.ascontiguousarray(et.transpose(0, 3, 1, 2))  # [n,8,20,20]
    out = np.zeros((n, 9, 8, PW, PW), BF_NP)
    for ky in range(3):
        for kx in range(3):
            py0, py1 = max(1, 2 - ky), min(21, 22 - ky)
            px0, px1 = max(1, 2 - kx), min(21, 22 - kx)
            y0, x0 = py0 + ky - 2, px0 + kx - 2
            out[:, ky * 3 + kx, :, py0:py1, px0:px1] = \
                et[:, :, y0:y0 + py1 - py0, x0:x0 + px1 - px0]
    return out.reshape(n, 72, NPAD)


def prep_shared(inputs):
    f = np.float32
    t = {}
    emb = np.asarray(inputs["emb_table"], f)
    norms = np.linalg.norm(emb, axis=1, keepdims=True)
    tbl = emb * np.where(norms > 1.0, f(1.0) / (norms + f(1e-7)), f(1.0))

    # conv0-e weights: wE0[(ky*3+kx)*8+e, o] = ec0_w[o,e,ky,kx]
    e0 = np.asarray(inputs["ec0_w"], f)                   # [32,8,3,3]
    t["wE0"] = np.ascontiguousarray(
        e0.transpose(2, 3, 1, 0).reshape(72, 32))

    e1 = np.asarray(inputs["ec1_w"], f)                   # [64,32,3,3]
    wE1 = np.zeros((96, 3, 64), f)
    for ky in range(3):
        for kx in range(3):
            wE1[ky * 32:(ky + 1) * 32, kx, :] = e1[:, :, ky, kx].T
    t["wE1"] = wE1

    wA0 = np.zeros((128, 9, 64), f)
    wB0 = np.zeros((128, 9, 64), f)
    wA1 = np.zeros((128, 9, 128), f)
    wB1 = np.zeros((128, 9, 128), f)
    for phi, p in enumerate(("p1", "p2", "p3")):
        c0 = np.asarray(inputs[p + "c0_w"], f)            # [64,64,3,3]
        c1 = np.asarray(inputs[p + "c1_w"], f)            # [128,64,3,3]
        for kx in range(3):
            wA0[0:64, phi * 3 + kx, :] = c0[:, :, 0, kx].T
            wA0[64:128, phi * 3 + kx, :] = c0[:, :, 1, kx].T
            wB0[64:128, phi * 3 + kx, :] = c0[:, :, 2, kx].T
            wA1[0:64, phi * 3 + kx, :] = c1[:, :, 0, kx].T
            wA1[64:128, phi * 3 + kx, :] = c1[:, :, 1, kx].T
            wB1[64:128, phi * 3 + kx, :] = c1[:, :, 2, kx].T
    t["wA0"], t["wB0"], t["wA1"], t["wB1"] = wA0, wB0, wA1, wB1
    # row-half-swapped variants for odd images (alternating [dup|relu]
    # plane layout): ky1 weights in rows 0:64, ky0 in 64:128; ky2 in 0:64
    for nm, w in (("wA0", wA0), ("wB0", wB0), ("wA1", wA1), ("wB1", wB1)):
        t[nm + "sw"] = np.concatenate([w[64:128], w[0:64]], axis=0)

    t["b0"] = np.asarray(inputs["ec0_b"], f).reshape(32, 1)
    t["b1"] = np.asarray(inputs["ec1_b"], f).reshape(64, 1)
    t["bc0"] = np.stack([np.asarray(inputs[p + "c0_b"], f)
                         for p in ("p1", "p2", "p3")], axis=1)   # [64,3]
    t["bc1"] = np.stack([np.asarray(inputs[p + "c1_b"], f)
                         for p in ("p1", "p2", "p3")], axis=1)   # [128,3]

    # big linear weights: [NWT, 128, W_PIX, 512]; fp8 phis prescaled
    NWT = NPIX // W_PIX
    for phi, p in enumerate(("p1", "p2", "p3")):
        w = np.asarray(inputs[p + "l_w"], f).reshape(512, 128, NWT, W_PIX)
        w = np.ascontiguousarray(w.transpose(2, 1, 3, 0))
        if phi in W8:
            t[f"wl{phi}"] = (w * F8_SCALE).astype(F8_NP)
        else:
            t[f"wl{phi}"] = w.astype(BF_NP)
    t["blpk"] = np.stack([np.asarray(inputs[p + "l_b"], f).reshape(4, 128).T
                          for p in ("p1", "p2", "p3")], axis=1)  # [128,3,4]

    for n in ("fc0", "fc1", "p3f0", "p3f1"):
        w = np.asarray(inputs[n + "_w"], f)               # [512,512] out,in
        t[n + "T"] = np.ascontiguousarray(
            w.T.reshape(4, 128, 512).transpose(1, 0, 2)).astype(BF_NP)
        t[n + "b"] = np.asarray(inputs[n + "_b"], f).reshape(4, 128).T

    z = np.asarray(inputs["z_vec"], f)                    # [64,512]
    t["zT"] = np.ascontiguousarray(
        z.T.reshape(4, 128, 64).transpose(1, 0, 2)).astype(BF_NP)
    t["zrow"] = z.astype(BF_NP)
    t["zsq"] = (z.astype(f) ** 2).sum(axis=1, keepdims=True).astype(f)
    t["esc"] = np.exp(np.asarray(inputs["scale"], f)).reshape(1, 1)
    for k in ("wE0", "wE1", "wA0", "wB0", "wA1", "wB1"):
        t[k] = t[k].astype(BF_NP)
    t["_tbl"] = tbl
    return t


def make_in_maps(inputs, BL):
    shared = prep_shared(inputs)
    tbl = shared.pop("_tbl")
    s = np.asarray(inputs["s"])
    sp = np.asarray(inputs["s_prime"])
    maps = []
    for c in range(N_CORES):
        m = dict(shared)
        m["embp_s"] = _embplanes(s[c * BL:(c + 1) * BL], tbl)
        m["embp_sp"] = _embplanes(sp[c * BL:(c + 1) * BL], tbl)
        maps.append(m)
    return maps


# --------------------------------------------------------------------------
# cached runner (jit once, reuse across calls)
# --------------------------------------------------------------------------

class Runner:
    def __init__(self, BL, nc=None):
        import jax
        from jax.sharding import Mesh, PartitionSpec, NamedSharding
        from jax.experimental.shard_map import shard_map
        from concourse import bass2jax
        self.BL = BL
        self.nc = nc if nc is not None else build_program(BL)
        nc = self.nc
        bass2jax.install_neuronx_cc_hook()

        partition_name = (nc.partition_id_tensor.name
                          if nc.partition_id_tensor else None)
        in_names, out_names, out_avals, zero_outs = [], [], [], []
        for alloc in nc.m.functions[0].allocations:
            if not isinstance(alloc, mybir.MemoryLocationSet):
                continue
            name = alloc.memorylocations[0].name
            if alloc.kind == "ExternalInput":
                if name != partition_name:
                    in_names.append(name)
            elif alloc.kind == "ExternalOutput":
                shape = tuple(alloc.tensor_shape)
                dtype = mybir.dt.np(alloc.dtype)
                out_names.append(name)
                out_avals.append(jax.core.ShapedArray(shape, dtype))
                zero_outs.append(np.zeros(shape, dtype))
        self.in_names, self.out_names = list(in_names), out_names
        self.out_avals, self.zero_outs = out_avals, zero_outs
        n_params, n_outs = len(in_names), len(out_avals)
        all_in_names = in_names + out_names
        if partition_name is not None:
            all_in_names = all_in_names + [partition_name]

        def _body(*args):
            operands = list(args)
            if partition_name is not None:
                operands.append(bass2jax.partition_id_tensor())
            return tuple(bass2jax._bass_exec_p.bind(
                *operands,
                out_avals=tuple(out_avals),
                in_names=tuple(all_in_names),
                out_names=tuple(out_names),
                lowering_input_output_aliases=(),
                sim_require_finite=True,
                sim_require_nnan=True,
                nc=nc,
            ))

        devices = jax.devices()[:N_CORES]
        self.mesh = Mesh(np.asarray(devices), ("core",))
        in_specs = (PartitionSpec("core"),) * (n_params + n_outs)
        out_specs = (PartitionSpec("core"),) * n_outs
        self.sharding = NamedSharding(self.mesh, PartitionSpec("core"))
        self.jitted = jax.jit(
            shard_map(_body, mesh=self.mesh, in_specs=in_specs,
                      out_specs=out_specs, check_rep=False),
            donate_argnums=tuple(range(n_params, n_params + n_outs)),
            keep_unused=True)
        self._staged = None
        self._jax = jax

    def stage(self, in_maps):
        """device_put the concatenated inputs once."""
        jax = self._jax
        concat = [np.concatenate([np.asarray(m[n]) for m in in_maps], axis=0)
                  for n in self.in_names]
        self._staged = [jax.device_put(a, self.sharding) for a in concat]
        jax.block_until_ready(self._staged)

    def run(self):
        jax = self._jax
        zo = [jax.device_put(
            np.zeros((N_CORES * z.shape[0], *z.shape[1:]), z.dtype),
            self.sharding) for z in self.zero_outs]
        jax.block_until_ready(zo)
        outs = self.jitted(*self._staged, *zo)
        jax.block_until_ready(outs)
        return outs

    def output(self, outs):
        o = np.asarray(outs[self.out_names.index("out")])
        return o  # [N_CORES*BL, N_CORES*BL]


_RUNNER_CACHE = {}


def _get_runner(BL):
    if BL not in _RUNNER_CACHE:
        _RUNNER_CACHE[BL] = Runner(BL)
    return _RUNNER_CACHE[BL]


def kernel(**inputs):
    assert int(np.asarray(inputs["downscale_factor"])) == 1
    BL = np.asarray(inputs["s"]).shape[0] // N_CORES
    r = _get_runner(BL)
    r.stage(make_in_maps(inputs, BL))
    outs = r.run()
    return r.output(outs).astype(np.float32)



# revision 28
# speedup vs baseline: 2.4898x; 1.6954x over previous
"""Trainium2 Bass kernel for nn_DSSMReverse (DSSM embed/conv/VQ/Gram model).

Strategy: data-parallel over batch across 8 NeuronCores (128 images each).
 - Embedding+conv0 are fully composed on host: 9-tap-shifted embedding
   planes (72 = 8ch x 9 taps) make conv0 a single K=72 matmul per image.
 - Remaining 3x3 convs run as shifted-window matmuls with row-group
   packing (K=96/128+64). E0 is 4-way col-tiled ([s_i0|sp_i0|s_i1|sp_i1]
   concurrent); c0 is image-pair col-tiled (i0 at psum 0:64/col 0, i1 at
   64:128/col 64, A tiles (0,c) + B tiles (64,c)). Relu copies (PSUM
   reads) stay on ACT/DVE; the row-shift duplicates are flat contiguous
   SBUF->SBUF DMAs (dst[0:462] <- src[22:484], pad zeros carry along),
   freeing Pool. NOTE: phase-A MM rate is ~1.2GHz here (activity/power
   clock-gate, see memory trn2-pe-throttle-findings), so fewer
   serial MM slots is the main phase-A lever.
 - phi conv outputs [128c x 400px] spill to DRAM in pixel-group-major
   layout; the 51200->512 linears stream x tiles [128,128,100] and
   weight tiles [128,10,512] (both contiguous) from HBM. Weights can be
   fp8-e4m3 per phi (scale 128, descaled in the bias epilogue).
 - Phase C (feat-major [4x128, B], fp16): VQ codebook argmax+gather via
   matmuls + PE transposes, fc stacks, L2 normalize via ones-matmul,
   AllGather of sp_out, per-core [128, 1024] block of the Gram matrix.
   Order: phi2 linear -> sp fc stack -> AllGather kick, then phi1 +
   quantize, then phi0 — the ~174us collective hides behind the phi1/
   phi0 weight streams instead of serializing before the gram block.
"""

import os
import numpy as np
import ml_dtypes

BF_NP = np.float16
F8_NP = ml_dtypes.float8_e4m3

import concourse.bacc as bacc
import concourse.bass as bass
import concourse.mybir as mybir
import concourse.tile as tile
from concourse.masks import make_identity

N_CORES = 8
B_FULL = 1024
H = W = 20
PW = 22            # padded plane width
NPIX = 400
NPAD = 484         # 22*22
EPS = 1e-4
F32 = mybir.dt.float32
BF16 = mybir.dt.float16    # 16-bit matmul dtype (fp16: 1 cyc/row)
FP8 = mybir.dt.float8e4
AF = mybir.ActivationFunctionType
ALU = mybir.AluOpType

W8 = (0,)          # phis whose big-linear weights are fp8 (0=p1,1=p2,2=p3)
                   # p1 is safe (2e-3, VQ-diluted); p2 flips VQ indices
                   # (catastrophic); p3 costs ~1.6e-2 (thin margin)
X8 = (0,)          # p1 x spills fp8 (phase B shares DMA bw: fewer bytes
                   # shortens the whole phase); was: fp8-x only pays on a
                   # DMA-bound phi, and the only fp8-safe phi (p1) is
                   # already PE-bound
F8_SCALE = 128.0
W_PIX = 10         # pixels per streamed weight tile in phase B
G_PIX = 100        # pixels per x tile in phase B


# --------------------------------------------------------------------------
# device program
# --------------------------------------------------------------------------

_STAGE_LOG = []            # (label, first-instruction-name) when tracing


def build_program(BL, phases="ABC", ib=2, sim1=False, trace_stages=False):
    nc = bacc.Bacc("TRN2", target_bir_lowering=False, debug=False,
                   num_devices=1 if sim1 else N_CORES)
    _STAGE_LOG.clear()

    def mark(label):
        if trace_stages:
            _STAGE_LOG.append((label, nc.get_next_instruction_name()))

    def inp(name, shape, dt=F32):
        return nc.dram_tensor(name, shape, dt, kind="ExternalInput").ap()

    embp_s = inp("embp_s", [BL, 72, NPAD], BF16)
    embp_sp = inp("embp_sp", [BL, 72, NPAD], BF16)
    wE0 = inp("wE0", [72, 32], BF16)
    wE1 = inp("wE1", [96, 3, 64], BF16)
    wA0 = inp("wA0", [128, 9, 64], BF16)
    wB0 = inp("wB0", [128, 9, 64], BF16)
    wA1 = inp("wA1", [128, 9, 128], BF16)
    wB1 = inp("wB1", [128, 9, 128], BF16)
    wA0w = inp("wA0sw", [128, 9, 64], BF16)
    wB0w = inp("wB0sw", [128, 9, 64], BF16)
    wA1w = inp("wA1sw", [128, 9, 128], BF16)
    wB1w = inp("wB1sw", [128, 9, 128], BF16)
    b0 = inp("b0", [32, 1])
    b1 = inp("b1", [64, 1])
    bc0 = inp("bc0", [64, 3])
    bc1 = inp("bc1", [128, 3])
    NWT = NPIX // W_PIX
    wl = [inp(f"wl{p}", [NWT, 128, W_PIX, 512], FP8 if p in W8 else BF16)
          for p in range(3)]
    blpk = inp("blpk", [128, 3, 4])
    fcT = {n: inp(n + "T", [128, 4, 512], BF16)
           for n in ("fc0", "fc1", "p3f0", "p3f1")}
    fcB = {n: inp(n + "b", [128, 4]) for n in ("fc0", "fc1", "p3f0", "p3f1")}
    zT = inp("zT", [128, 4, 64], BF16)
    zrow = inp("zrow", [64, 512], BF16)
    zsq = inp("zsq", [64, 1])
    esc = inp("esc", [1, 1])

    out_d = nc.dram_tensor("out", [BL, N_CORES * BL], F32,
                           kind="ExternalOutput").ap()

    cpc = min(N_CORES, 512 // BL)          # gram col-cores per matmul chunk
    n_chunks = (N_CORES + cpc - 1) // cpc
    NG = NPIX // G_PIX                     # x pixel groups in phase B

    with tile.TileContext(nc) as tc:
        with (
            tc.tile_pool(name="const", bufs=1) as cst,
            tc.tile_pool(name="inter", bufs=1) as inter,
            tc.tile_pool(name="dram", bufs=1, space="DRAM") as dram,
        ):
            # ---- persistent weights -> SBUF
            def load(ap, shape, tag, dt=F32):
                t = cst.tile(shape, dt, tag=tag, name=tag)
                nc.sync.dma_start(out=t[:], in_=ap[:])
                return t

            wE0_s = load(wE0, [72, 32], "wE0", BF16)
            wE1_s = load(wE1, [96, 3, 64], "wE1", BF16)
            wA0_s = load(wA0, [128, 9, 64], "wA0", BF16)
            wB0_s = load(wB0, [128, 9, 64], "wB0", BF16)
            wA1_s = load(wA1, [128, 9, 128], "wA1", BF16)
            wB1_s = load(wB1, [128, 9, 128], "wB1", BF16)
            # swapped-weight variants (wA0sw..wB1sw) stay in DRAM: they
            # are only needed by the (currently disabled) c1 B-row
            # pairing path, and the const pool has no SBUF headroom
            del wA0w, wB0w, wA1w, wB1w
            b0_s = load(b0, [32, 1], "b0")
            b1_s = load(b1, [64, 1], "b1")
            bc0_s = load(bc0, [64, 3], "bc0")
            bc1_s = load(bc1, [128, 3], "bc1")
            bl_s = load(blpk, [128, 3, 4], "blpk")
            # phase-C-only weights: demote so they don't block the first
            # embed-plane DMAs on the sync queue at kernel start
            with tc.high_priority(offset=-100000):
                fcT_s = {n: load(fcT[n], [128, 4, 512], n + "T", BF16)
                         for n in fcT}
                fcB_s = {n: load(fcB[n], [128, 4], n + "b") for n in fcB}
                zT_s = load(zT, [128, 4, 64], "zT", BF16)
                zrow_s = load(zrow, [64, 512], "zrow", BF16)
                zsq_s = load(zsq, [64, 1], "zsq")
                esc_s = load(esc, [1, 1], "esc")

            ident = cst.tile([128, 128], F32, tag="ident")
            make_identity(nc, ident[:])
            ones_col = cst.tile([128, 1], F32, tag="ones_col")
            nc.vector.memset(ones_col[:], 1.0)
            ones_row = cst.tile([1, 128], F32, tag="ones_row")
            nc.vector.memset(ones_row[:], 1.0)

            # DRAM spill for phi conv1 outputs, [c, pxgroup, img, px_in_g]
            xdt = [FP8 if i in X8 else BF16 for i in range(3)]
            xsp = [dram.tile([128, NG, BL, G_PIX], xdt[i], tag=f"xsp{i}",
                             name=f"xsp{i}") for i in range(3)]
            # collective buffers
            spT_d = dram.tile([128, 4, BL], BF16, tag="spT")
            spall = dram.tile([N_CORES, 128, 4, BL], BF16, tag="spall",
                              **({} if sim1 else {"addr_space": "Shared"}))

            # ============================================================
            # phase A: conv stacks, IB images per instruction group (fp16)
            # ============================================================
            IB = ib if BL % ib == 0 else 1
            with (
                tc.tile_pool(name="pa", bufs=2) as pa,
                tc.tile_pool(name="papsh", bufs=2, space="PSUM") as papsh,
                tc.tile_pool(name="papsb", bufs=2, space="PSUM") as papsb,
            ):
                def win(t4, i, y0, y1, kx):
                    """conv window of image i: [P, 20, 20] at row y0..y1-1."""
                    v = t4[:, i, :].rearrange("p (y x) -> p y x", y=PW, x=PW)
                    return v[:, y0:y1, kx:kx + 20]

                # rotating engine choice for relu copies (Pool is ~1.3x
                # slower per element, so it gets a smaller share)
                ENG = (nc.scalar, nc.vector, nc.gpsimd)

                def relu_copy(eng, dst, src, bias):
                    if eng is nc.scalar:
                        eng.activation(dst, src, AF.Relu, bias=bias)
                    else:
                        eng.tensor_scalar(dst, src, bias, 0.0, ALU.add,
                                          ALU.max)

                def dup_copy(dst, src):
                    # Pool cannot read PSUM: shifted duplicates are plain
                    # SBUF->SBUF copies of already-relu'd data
                    nc.gpsimd.tensor_copy(dst, src)

                stash = {}   # group idx -> dict of live tiles

                def dma_head(g):
                    mark(f'dma_head:{g}')
                    st = stash.setdefault(g, {})
                    g0 = g * IB
                    for m, ap in (("s", embp_s), ("sp", embp_sp)):
                        t = pa.tile([72, IB, NPAD], BF16, tag=f"embp_{m}",
                                    bufs=3)
                        nc.sync.dma_start(
                            out=t[:],
                            in_=ap[g0:g0 + IB].rearrange("i p n -> p i n"))
                        st[f"embp_{m}"] = t

                def head_e0(g):
                    # E0 matmuls (K=72, 9 taps host-composed): 4-way
                    # col-tiled concurrent pack [s_i0|sp_i0|s_i1|sp_i1],
                    # then relu + 2 flat row-shift DMA dups per tensor
                    mark(f'head_e0:{g}')
                    st = stash[g]
                    g0 = g * IB
                    ps = papsh.tile([128, IB, 512], F32, tag="ps")
                    for i in range(IB):
                        for mi, m in enumerate(("s", "sp")):
                            q = mi + 2 * i
                            v = st[f"embp_{m}"][:, i, :].rearrange(
                                "p (y x) -> p y x", y=PW, x=PW)
                            nc.tensor.matmul(
                                ps[q * 32:(q + 1) * 32, 0, 0:NPIX],
                                wE0_s[:], v[:, 1:21, 1:21],
                                start=True, stop=True,
                                tile_position=(0, q * 32))
                    for mi, m in enumerate(("s", "sp")):
                        eb = pa.tile([96, IB, NPAD], BF16, tag="e0big",
                                     bufs=3)
                        if g0 < 3 * IB:
                            nc.gpsimd.memset(eb[:], 0.0)
                        ev = eb[:].rearrange("p i (y x) -> p i y x",
                                             y=PW, x=PW)
                        for i in range(IB):
                            q = mi + 2 * i
                            pv = ps[q * 32:(q + 1) * 32, 0,
                                    0:NPIX].rearrange(
                                "p (y x) -> p y x", y=20, x=20)
                            relu_copy((nc.scalar, nc.vector)[i % 2],
                                      ev[0:32, i, 1:21, 1:21], pv[:],
                                      b0_s[:])
                        # row-shifted duplicates: flat contiguous DMA
                        # (pad cols/rows carry zeros along correctly)
                        nc.gpsimd.dma_start(
                            out=eb[32:64, :, 0:462],
                            in_=eb[0:32, :, 22:484])
                        nc.gpsimd.dma_start(
                            out=eb[64:96, :, 0:440],
                            in_=eb[0:32, :, 44:484])
                        st[f"e0big_{m}"] = eb

                def head_e1(g):
                    # E1 matmuls (K=96, 3 kx taps) both tensors into one
                    # psum [128,...] + embbig relus + diff
                    mark(f'head_e1:{g}')
                    st = stash[g]
                    g0 = g * IB
                    ps = papsh.tile([128, IB, 512], F32, tag="ps")
                    for mi, m in enumerate(("s", "sp")):
                        for kx in range(3):
                            for i in range(IB):
                                nc.tensor.matmul(
                                    ps[mi * 64:(mi + 1) * 64, i, 0:NPIX],
                                    wE1_s[:, kx, :],
                                    win(st[f"e0big_{m}"], i, 0, 20, kx),
                                    start=(kx == 0), stop=(kx == 2))
                    for mi, m in enumerate(("s", "sp")):
                        eb = pa.tile([128, IB, NPAD], BF16, tag="emb",
                                     bufs=4)
                        if g0 < 4 * IB:
                            nc.gpsimd.memset(eb[:], 0.0)
                        ebv = eb[:].rearrange("p i (y x) -> p i y x",
                                              y=PW, x=PW)
                        p1v = ps[mi * 64:(mi + 1) * 64, :, 0:NPIX].rearrange(
                            "p i (y x) -> p i y x", y=20, x=20)
                        relu_copy(nc.scalar if mi == 0 else nc.vector,
                                  ebv[0:64, :, 1:21, 1:21], p1v[:], b1_s[:])
                        nc.gpsimd.dma_start(
                            out=eb[64:128, :, 0:462],
                            in_=eb[0:64, :, 22:484])
                        st[f"emb_{m}"] = eb
                    diff = pa.tile([128, IB, NPAD], BF16, tag="diff",
                                   bufs=2)
                    nc.gpsimd.tensor_tensor(diff[:], st["emb_sp"][:],
                                            st["emb_s"][:], ALU.subtract)
                    st["diff"] = diff

                def c0_mm(g, phi):
                    # image-pair col-packed: i0 -> psum 0:64, i1 -> 64:128;
                    # A pairs at col 0/64, B pairs at (64, 0/64)
                    mark(f'c0_mm:{g}:{phi}')
                    st = stash[g]
                    src = {0: st["emb_s"], 1: st["diff"],
                           2: st["emb_sp"]}[phi]
                    ps = papsb.tile([128, IB // 2, 512], F32, tag="ps")
                    for kx in range(3):
                        for i in range(IB):
                            nc.tensor.matmul(
                                ps[64 * (i % 2):64 * (i % 2) + 64,
                                   i // 2, 0:NPIX],
                                wA0_s[:, phi * 3 + kx, :],
                                win(src, i, 0, 20, kx),
                                start=(kx == 0), stop=False,
                                tile_position=(0, 64 * (i % 2)))
                    for kx in range(3):
                        for i in range(IB):
                            nc.tensor.matmul(
                                ps[64 * (i % 2):64 * (i % 2) + 64,
                                   i // 2, 0:NPIX],
                                wB0_s[64:128, phi * 3 + kx, :],
                                win(src, i, 1, 21, kx)[64:128],
                                start=False, stop=(kx == 2),
                                tile_position=(64, 64 * (i % 2)))
                    return ps

                def c0_relu(g, phi, ps, k):
                    mark(f'c0_relu:{g}:{phi}')
                    st = stash[g]
                    cb = pa.tile([128, IB, NPAD], BF16, tag="c0big",
                                 bufs=3)
                    if g * IB < 3 * IB:
                        nc.gpsimd.memset(cb[:], 0.0)
                    cv = cb[:].rearrange("p i (y x) -> p i y x", y=PW, x=PW)
                    for i in range(IB):
                        c0v = ps[64 * (i % 2):64 * (i % 2) + 64, i // 2,
                                 0:NPIX].rearrange(
                            "p (y x) -> p y x", y=20, x=20)
                        relu_copy(ENG[(k + i) % 2], cv[0:64, i, 1:21, 1:21],
                                  c0v[:], bc0_s[:, phi:phi + 1])
                    nc.gpsimd.dma_start(
                        out=cb[64:128, :, 0:462],
                        in_=cb[0:64, :, 22:484])
                    st[f"c0big{phi}"] = cb

                def c1_mm(g, phi):
                    # A full-array serial; B K=64 row-tiled pairs (even at
                    # rows 64:128, odd at rows 0:64 via swapped weights)
                    mark(f'c1_mm:{g}:{phi}')
                    st = stash[g]
                    src = st[f"c0big{phi}"]
                    ps = papsb.tile([128, IB, 512], F32, tag="ps")
                    for i in range(IB):
                        for kx in range(3):
                            nc.tensor.matmul(
                                ps[:, i, 0:NPIX],
                                wA1_s[:, phi * 3 + kx, :],
                                win(src, i, 0, 20, kx),
                                start=(kx == 0), stop=False)
                        for kx in range(3):
                            nc.tensor.matmul(
                                ps[:, i, 0:NPIX],
                                wB1_s[64:128, phi * 3 + kx, :],
                                win(src, i, 1, 21, kx)[64:128],
                                start=False, stop=(kx == 2))
                    st[f"psC1_{phi}"] = ps

                def c1_out(g, phi, k):
                    mark(f'c1_out:{g}:{phi}')
                    st = stash[g]
                    g0 = g * IB
                    c1sb = pa.tile([128, NG, IB, G_PIX], xdt[phi],
                                   tag=f"c1sb{xdt[phi]}", bufs=3)
                    # spill copies are off the critical path: demote them
                    # so chain-critical relu copies win engine races
                    with tc.high_priority(offset=-100):
                        for i in range(IB):
                            src = st[f"psC1_{phi}"][:, i, 0:NPIX].rearrange(
                                "p (g q) -> p g q", g=NG, q=G_PIX)
                            relu_copy(ENG[(k + i) % 2], c1sb[:, :, i, :],
                                      src, bc1_s[:, phi:phi + 1])
                        nc.sync.dma_start(
                            out=xsp[phi][:, :, g0:g0 + IB, :],
                            in_=c1sb[:])

                NGRP = BL // IB if "A" in phases else 0
                if NGRP:
                    dma_head(0)
                    head_e0(0)
                    head_e1(0)
                for g in range(NGRP):
                    nxt = g + 1 if g + 1 < NGRP else None
                    if nxt is not None:
                        with tc.high_priority():
                            dma_head(nxt)
                    # body: c0P1, c0P3, c1P1, c0P2 (+ relus); spill copies
                    # (c1_out) are demoted below the next group's head
                    # copies so the chain-critical embed junction never
                    # queues behind them.
                    ps0 = c0_mm(g, 0)
                    c0_relu(g, 0, ps0, 0)
                    ps2 = c0_mm(g, 2)
                    c0_relu(g, 2, ps2, 1)
                    c1_mm(g, 0)
                    ps1 = c0_mm(g, 1)
                    c0_relu(g, 1, ps1, 2)
                    if nxt is not None:
                        with tc.high_priority():
                            head_e0(nxt)
                    c1_out(g, 0, 2)
                    c1_mm(g, 2)
                    if nxt is not None:
                        with tc.high_priority():
                            head_e1(nxt)
                    c1_out(g, 2, 0)
                    c1_mm(g, 1)
                    c1_out(g, 1, 1)
                    stash.pop(g, None)

            # ============================================================
            # phase B: 51200->512 linears (+ transpose to feat-major fp16)
            # phase C interleaved: quantize after phi=1, fc stacks after
            # ============================================================
            xT = {}      # feat-major [128, 4, BL] fp16 phi outputs (+bias)

            with (
                tc.tile_pool(name="pb", bufs=2) as pb,
                tc.tile_pool(name="pbps", bufs=2, space="PSUM") as pbps,
                tc.tile_pool(name="pc", bufs=1) as pc,
                tc.tile_pool(name="pcps", bufs=4, space="PSUM") as pcps,
            ):
                def linear_phi(phi):
                    mark(f'linear_phi:{phi}')
                    wdt = FP8 if phi in W8 else BF16
                    acc = pbps.tile([BL, 512], F32, tag="acc")
                    for g in range(NG):
                        xg = pb.tile([128, BL, G_PIX], xdt[phi],
                                     bufs=2,
                                     tag=f"xg{xdt[phi]}")
                        nc.sync.dma_start(out=xg[:], in_=xsp[phi][:, g])
                        for t in range(G_PIX // W_PIX):
                            wt = pb.tile([128, W_PIX, 512], wdt,
                                         bufs=4,
                                         tag=f"wt{wdt}")
                            _eng = (nc.sync, nc.scalar, nc.gpsimd)[t % 3]
                            _eng.dma_start(
                                out=wt[:],
                                in_=wl[phi][g * (G_PIX // W_PIX) + t])
                            for j in range(W_PIX):
                                gp = g * G_PIX + t * W_PIX + j
                                nc.tensor.matmul(
                                    acc[:], xg[:, :, t * W_PIX + j],
                                    wt[:, j, :],
                                    start=(gp == 0), stop=(gp == NPIX - 1))
                    # PSUM [BL, 512] -> SBUF, transpose to [128, 4, BL]
                    # + bias (+ fp8 descale)
                    asb = pc.tile([BL, 512], F32, tag=f"asb{phi}")
                    nc.scalar.copy(asb[:], acc[:])
                    t = inter.tile([128, 4, BL], BF16, tag=f"xT{phi}")
                    sphi = 1.0 / F8_SCALE if phi in W8 else 1.0
                    for k in range(4):
                        pt = pcps.tile([128, 512], F32, tag="ps")
                        nc.tensor.transpose(pt[:, 0:BL],
                                            asb[:, k * 128:(k + 1) * 128],
                                            ident[0:BL, 0:BL])
                        nc.scalar.activation(t[:, k, :], pt[:, 0:BL],
                                             AF.Identity,
                                             bias=bl_s[:, phi, k:k + 1],
                                             scale=sphi)
                    xT[phi] = t

                def fc_layer(h_in, wname, relu, tag):
                    mark(f'fc_layer:{wname}')
                    h_out = pc.tile([128, 4, BL], BF16, tag=tag)
                    for j in range(4):
                        ps = pcps.tile([128, 512], F32, tag="ps")
                        for k in range(4):
                            nc.tensor.matmul(
                                ps[:, 0:BL],
                                fcT_s[wname][:, k, j * 128:(j + 1) * 128],
                                h_in[:, k, :],
                                start=(k == 0), stop=(k == 3))
                        if relu:
                            nc.vector.tensor_scalar(
                                h_out[:, j, :], ps[:, 0:BL],
                                fcB_s[wname][:, j:j + 1], 0.0, ALU.add,
                                ALU.max)
                        else:
                            nc.vector.tensor_scalar(
                                h_out[:, j, :], ps[:, 0:BL],
                                fcB_s[wname][:, j:j + 1], None, ALU.add)
                    return h_out

                def normalize(h_in, with_escale, tag):
                    mark(f'normalize:{tag}')
                    # returns h_in * 1/(||h||+eps) [* exp(scale)]
                    sq = pc.tile([128, 4, BL], F32, tag=tag + "_sq")
                    nc.vector.tensor_tensor(sq[:], h_in[:], h_in[:],
                                            ALU.mult)
                    pn = pcps.tile([128, 512], F32, tag="ps")
                    for k in range(4):
                        nc.tensor.matmul(pn[0:1, 0:BL], ones_col[:],
                                         sq[:, k, :],
                                         start=(k == 0), stop=(k == 3))
                    tn = pc.tile([1, BL], F32, tag=tag + "_tn")
                    nc.scalar.activation(tn[:], pn[0:1, 0:BL], AF.Sqrt)
                    nc.vector.tensor_scalar_add(tn[:], tn[:], EPS)
                    rn = pc.tile([1, BL], F32, tag=tag + "_rn")
                    nc.vector.reciprocal(rn[:], tn[:])
                    if with_escale:
                        nc.vector.tensor_scalar_mul(rn[:], rn[:], esc_s[:])
                    pbx = pcps.tile([128, 512], F32, tag="ps")
                    nc.tensor.matmul(pbx[:, 0:BL], ones_row[:], rn[:],
                                     start=True, stop=True)
                    h_out = pc.tile([128, 4, BL], BF16, tag=tag)
                    for k in range(4):
                        nc.vector.tensor_tensor(h_out[:, k, :],
                                                h_in[:, k, :],
                                                pbx[:, 0:BL], ALU.mult)
                    return h_out

                def _dummy_out():
                    dummy = pc.tile([BL, N_CORES * BL], F32, tag="dummy")
                    nc.vector.memset(dummy[:], 0.0)
                    nc.sync.dma_start(out=out_d[:], in_=dummy[:])

                def _bc():
                    if "B" not in phases:
                        _dummy_out()
                        return
                    # ---- sp path FIRST: its fc stack + normalize feed the
                    # AllGather, which then hides behind the p2/p1 streams
                    linear_phi(2)

                    if "C" not in phases:
                        linear_phi(1)
                        linear_phi(0)
                        _dummy_out()
                        return

                    g1 = fc_layer(xT[2], "p3f0", True, "g1")
                    g2 = fc_layer(g1, "p3f1", False, "g2")
                    sp_outT = normalize(g2, False, "spoT")
                    mark('allgather')
                    nc.sync.dma_start(out=spT_d[:], in_=sp_outT[:])
                    if sim1:
                        for c in range(N_CORES):
                            nc.sync.dma_start(out=spall[c], in_=spT_d[:])
                    else:
                        nc.gpsimd.collective_compute(
                            "AllGather", ALU.bypass,
                            replica_groups=[list(range(N_CORES))],
                            ins=[spT_d[:]], outs=[spall[:]])

                    # ---- diff path linear + quantize
                    linear_phi(1)

                    # quantize: scoreT[j,b] = zsq_j - 2 * (z @ diff)[j,b]
                    mark('quantize')
                    pG = pcps.tile([128, 512], F32, tag="ps")
                    for k in range(4):
                        nc.tensor.matmul(pG[0:64, 0:BL], zT_s[:, k, :],
                                         xT[1][:, k, :],
                                         start=(k == 0), stop=(k == 3))
                    scT = pc.tile([64, BL], F32, tag="scT")
                    nc.scalar.activation(scT[:], pG[0:64, 0:BL], AF.Identity,
                                         bias=zsq_s[:], scale=-2.0)
                    pSc = pcps.tile([128, 512], F32, tag="ps")
                    nc.tensor.transpose(pSc[0:BL, 0:64], scT[:],
                                        ident[0:64, 0:64])
                    scB = pc.tile([BL, 64], F32, tag="scB")
                    nc.vector.tensor_copy(scB[:], pSc[0:BL, 0:64])
                    mx = pc.tile([BL, 1], F32, tag="mx")
                    nc.vector.tensor_reduce(mx[:], scB[:],
                                            mybir.AxisListType.X, ALU.max)
                    ohB = pc.tile([BL, 64], F32, tag="ohB")
                    nc.vector.tensor_scalar(ohB[:], scB[:], mx[:], None,
                                            ALU.is_ge)
                    pOh = pcps.tile([128, 512], F32, tag="ps")
                    nc.tensor.transpose(pOh[0:64, 0:BL], ohB[:],
                                        ident[0:BL, 0:BL])
                    ohT = pc.tile([64, BL], BF16, tag="ohT")
                    nc.vector.tensor_copy(ohT[:], pOh[0:64, 0:BL])

                    # ---- s path linear
                    linear_phi(0)

                    # z_matrix gather + add s_int
                    mark('zgather')
                    h0 = pc.tile([128, 4, BL], BF16, tag="h0")
                    for k in range(4):
                        pz = pcps.tile([128, 512], F32, tag="ps")
                        nc.tensor.matmul(pz[:, 0:BL],
                                         zrow_s[:, k * 128:(k + 1) * 128],
                                         ohT[:], start=True, stop=True)
                        nc.vector.scalar_tensor_tensor(
                            h0[:, k, :], pz[:, 0:BL], 0.0, xT[0][:, k, :],
                            ALU.bypass, ALU.add)

                    h1 = fc_layer(h0, "fc0", True, "h1")
                    h2 = fc_layer(h1, "fc1", False, "h2")
                    s_outT = normalize(h2, True, "soT")

                    # ---- gram block: out[my_b, all_b]
                    mark('gram')
                    outsb = pc.tile([BL, N_CORES * BL], F32, tag="outsb")
                    spv = spall[:].rearrange("c p k b -> p k c b")
                    for h in range(n_chunks):
                        ncol = cpc * BL
                        pi = pcps.tile([128, 512], F32, tag="ps")
                        for k in range(4):
                            sps = pb.tile([128, cpc, BL], BF16, tag="sps")
                            nc.sync.dma_start(
                                out=sps[:],
                                in_=spv[:, k, h * cpc:(h + 1) * cpc, :])
                            spsf = sps[:].rearrange("p c b -> p (c b)")
                            nc.tensor.matmul(pi[0:BL, 0:ncol],
                                             s_outT[:, k, :], spsf,
                                             start=(k == 0), stop=(k == 3))
                        nc.scalar.copy(outsb[:, h * ncol:(h + 1) * ncol],
                                       pi[0:BL, 0:ncol])
                    nc.sync.dma_start(out=out_d[:], in_=outsb[:])

                _bc()

    nc.finalize()
    return nc


# --------------------------------------------------------------------------
# host-side input preparation
# --------------------------------------------------------------------------

def _embplanes(idx, tbl):
    """[n,20,20] int -> [n,72,484] f16: 9-tap-shifted embedding planes."""
    n = idx.shape[0]
    et = tbl[idx]                                    # [n,20,20,8]
    et = np.ascontiguousarray(et.transpose(0, 3, 1, 2))  # [n,8,20,20]
    out = np.zeros((n, 9, 8, PW, PW), BF_NP)
    for ky in range(3):
        for kx in range(3):
            py0, py1 = max(1, 2 - ky), min(21, 22 - ky)
            px0, px1 = max(1, 2 - kx), min(21, 22 - kx)
            y0, x0 = py0 + ky - 2, px0 + kx - 2
            out[:, ky * 3 + kx, :, py0:py1, px0:px1] = \
                et[:, :, y0:y0 + py1 - py0, x0:x0 + px1 - px0]
    return out.reshape(n, 72, NPAD)


def prep_shared(inputs):
    f = np.float32
    t = {}
    emb = np.asarray(inputs["emb_table"], f)
    norms = np.linalg.norm(emb, axis=1, keepdims=True)
    tbl = emb * np.where(norms > 1.0, f(1.0) / (norms + f(1e-7)), f(1.0))

    # conv0-e weights: wE0[(ky*3+kx)*8+e, o] = ec0_w[o,e,ky,kx]
    e0 = np.asarray(inputs["ec0_w"], f)                   # [32,8,3,3]
    t["wE0"] = np.ascontiguousarray(
        e0.transpose(2, 3, 1, 0).reshape(72, 32))

    e1 = np.asarray(inputs["ec1_w"], f)                   # [64,32,3,3]
    wE1 = np.zeros((96, 3, 64), f)
    for ky in range(3):
        for kx in range(3):
            wE1[ky * 32:(ky + 1) * 32, kx, :] = e1[:, :, ky, kx].T
    t["wE1"] = wE1

    wA0 = np.zeros((128, 9, 64), f)
    wB0 = np.zeros((128, 9, 64), f)
    wA1 = np.zeros((128, 9, 128), f)
    wB1 = np.zeros((128, 9, 128), f)
    for phi, p in enumerate(("p1", "p2", "p3")):
        c0 = np.asarray(inputs[p + "c0_w"], f)            # [64,64,3,3]
        c1 = np.asarray(inputs[p + "c1_w"], f)            # [128,64,3,3]
        for kx in range(3):
            wA0[0:64, phi * 3 + kx, :] = c0[:, :, 0, kx].T
            wA0[64:128, phi * 3 + kx, :] = c0[:, :, 1, kx].T
            wB0[64:128, phi * 3 + kx, :] = c0[:, :, 2, kx].T
            wA1[0:64, phi * 3 + kx, :] = c1[:, :, 0, kx].T
            wA1[64:128, phi * 3 + kx, :] = c1[:, :, 1, kx].T
            wB1[64:128, phi * 3 + kx, :] = c1[:, :, 2, kx].T
    t["wA0"], t["wB0"], t["wA1"], t["wB1"] = wA0, wB0, wA1, wB1
    # row-half-swapped variants for odd images (alternating [dup|relu]
    # plane layout): ky1 weights in rows 0:64, ky0 in 64:128; ky2 in 0:64
    for nm, w in (("wA0", wA0), ("wB0", wB0), ("wA1", wA1), ("wB1", wB1)):
        t[nm + "sw"] = np.concatenate([w[64:128], w[0:64]], axis=0)

    t["b0"] = np.asarray(inputs["ec0_b"], f).reshape(32, 1)
    t["b1"] = np.asarray(inputs["ec1_b"], f).reshape(64, 1)
    t["bc0"] = np.stack([np.asarray(inputs[p + "c0_b"], f)
                         for p in ("p1", "p2", "p3")], axis=1)   # [64,3]
    t["bc1"] = np.stack([np.asarray(inputs[p + "c1_b"], f)
                         for p in ("p1", "p2", "p3")], axis=1)   # [128,3]

    # big linear weights: [NWT, 128, W_PIX, 512]; fp8 phis prescaled
    NWT = NPIX // W_PIX
    for phi, p in enumerate(("p1", "p2", "p3")):
        w = np.asarray(inputs[p + "l_w"], f).reshape(512, 128, NWT, W_PIX)
        w = np.ascontiguousarray(w.transpose(2, 1, 3, 0))
        if phi in W8:
            t[f"wl{phi}"] = (w * F8_SCALE).astype(F8_NP)
        else:
            t[f"wl{phi}"] = w.astype(BF_NP)
    t["blpk"] = np.stack([np.asarray(inputs[p + "l_b"], f).reshape(4, 128).T
                          for p in ("p1", "p2", "p3")], axis=1)  # [128,3,4]

    for n in ("fc0", "fc1", "p3f0", "p3f1"):
        w = np.asarray(inputs[n + "_w"], f)               # [512,512] out,in
        t[n + "T"] = np.ascontiguousarray(
            w.T.reshape(4, 128, 512).transpose(1, 0, 2)).astype(BF_NP)
        t[n + "b"] = np.asarray(inputs[n + "_b"], f).reshape(4, 128).T

    z = np.asarray(inputs["z_vec"], f)                    # [64,512]
    t["zT"] = np.ascontiguousarray(
        z.T.reshape(4, 128, 64).transpose(1, 0, 2)).astype(BF_NP)
    t["zrow"] = z.astype(BF_NP)
    t["zsq"] = (z.astype(f) ** 2).sum(axis=1, keepdims=True).astype(f)
    t["esc"] = np.exp(np.asarray(inputs["scale"], f)).reshape(1, 1)
    for k in ("wE0", "wE1", "wA0", "wB0", "wA1", "wB1"):
        t[k] = t[k].astype(BF_NP)
    t["_tbl"] = tbl
    return t


def make_in_maps(inputs, BL):
    shared = prep_shared(inputs)
    tbl = shared.pop("_tbl")
    s = np.asarray(inputs["s"])
    sp = np.asarray(inputs["s_prime"])
    maps = []
    for c in range(N_CORES):
        m = dict(shared)
        m["embp_s"] = _embplanes(s[c * BL:(c + 1) * BL], tbl)
        m["embp_sp"] = _embplanes(sp[c * BL:(c + 1) * BL], tbl)
        maps.append(m)
    return maps


# --------------------------------------------------------------------------
# cached runner (jit once, reuse across calls)
# --------------------------------------------------------------------------

class Runner:
    def __init__(self, BL, nc=None):
        import jax
        from jax.sharding import Mesh, PartitionSpec, NamedSharding
        from jax.experimental.shard_map import shard_map
        from concourse import bass2jax
        self.BL = BL
        self.nc = nc if nc is not None else build_program(BL)
        nc = self.nc
        bass2jax.install_neuronx_cc_hook()

        partition_name = (nc.partition_id_tensor.name
                          if nc.partition_id_tensor else None)
        in_names, out_names, out_avals, zero_outs = [], [], [], []
        for alloc in nc.m.functions[0].allocations:
            if not isinstance(alloc, mybir.MemoryLocationSet):
                continue
            name = alloc.memorylocations[0].name
            if alloc.kind == "ExternalInput":
                if name != partition_name:
                    in_names.append(name)
            elif alloc.kind == "ExternalOutput":
                shape = tuple(alloc.tensor_shape)
                dtype = mybir.dt.np(alloc.dtype)
                out_names.append(name)
                out_avals.append(jax.core.ShapedArray(shape, dtype))
                zero_outs.append(np.zeros(shape, dtype))
        self.in_names, self.out_names = list(in_names), out_names
        self.out_avals, self.zero_outs = out_avals, zero_outs
        n_params, n_outs = len(in_names), len(out_avals)
        all_in_names = in_names + out_names
        if partition_name is not None:
            all_in_names = all_in_names + [partition_name]

        def _body(*args):
            operands = list(args)
            if partition_name is not None:
                operands.append(bass2jax.partition_id_tensor())
            return tuple(bass2jax._bass_exec_p.bind(
                *operands,
                out_avals=tuple(out_avals),
                in_names=tuple(all_in_names),
                out_names=tuple(out_names),
                lowering_input_output_aliases=(),
                sim_require_finite=True,
                sim_require_nnan=True,
                nc=nc,
            ))

        devices = jax.devices()[:N_CORES]
        self.mesh = Mesh(np.asarray(devices), ("core",))
        in_specs = (PartitionSpec("core"),) * (n_params + n_outs)
        out_specs = (PartitionSpec("core"),) * n_outs
        self.sharding = NamedSharding(self.mesh, PartitionSpec("core"))
        self.jitted = jax.jit(
            shard_map(_body, mesh=self.mesh, in_specs=in_specs,
                      out_specs=out_specs, check_rep=False),
            donate_argnums=tuple(range(n_params, n_params + n_outs)),
            keep_unused=True)
        self._staged = None
        self._jax = jax

    def stage(self, in_maps):
        """device_put the concatenated inputs once."""
        jax = self._jax
        concat = [np.concatenate([np.asarray(m[n]) for m in in_maps], axis=0)
                  for n in self.in_names]
        self._staged = [jax.device_put(a, self.sharding) for a in concat]
        jax.block_until_ready(self._staged)

    def run(self):
        jax = self._jax
        zo = [jax.device_put(
            np.zeros((N_CORES * z.shape[0], *z.shape[1:]), z.dtype),
            self.sharding) for z in self.zero_outs]
        jax.block_until_ready(zo)
        outs = self.jitted(*self._staged, *zo)
        jax.block_until_ready(outs)
        return outs

    def output(self, outs):
        o = np.asarray(outs[self.out_names.index("out")])
        return o  # [N_CORES*BL, N_CORES*BL]


_RUNNER_CACHE = {}


def _get_runner(BL):
    if BL not in _RUNNER_CACHE:
        _RUNNER_CACHE[BL] = Runner(BL)
    return _RUNNER_CACHE[BL]


def kernel(**inputs):
    assert int(np.asarray(inputs["downscale_factor"])) == 1
    BL = np.asarray(inputs["s"]).shape[0] // N_CORES
    r = _get_runner(BL)
    r.stage(make_in_maps(inputs, BL))
    outs = r.run()
    return r.output(outs).astype(np.float32)

